# revision 73
# baseline (speedup 1.0000x reference)
"""Bass/TRN2 kernel for nn_BilateralCostVolume — patch-gather scheme v2.

Sharding: core k handles batch b = k//4, output rows h in [20*(k%4), +20).
Per core, per pixel, per warp (F: +displacement on f2n; B: -displacement on
f1n) gather an 11x11 patch from a DRAM table laid out [xwin][y] so the 11
patch rows per pixel are CONTIGUOUS (one gather descriptor per pixel,
elem_step=640, elem_size=7040).  All 81 displacements are then computed
on-chip with static 3-tap separable interpolation (carry folded into
per-pixel weights), channel dot, mask.

Engine split (tunable tables below): y-pass fully on DVE (tensor_scalar
muls run in 4x perf mode, tensor_tensor adds in 2x, merged across the 9
units of a warp), x-pass muls mostly on ACT with adds on Pool (chunked so
they start while the ACT mul stream runs; stage-interleaved across warps
so no engine waits on another's adds), dot tree split DVE/Pool.  The dot
of row-block rb-1 is emitted inside rb's body (software pipelining) and
gathers are prefetched two row-blocks ahead.  Norm squares run on ACT and
the normalize-multiply on Pool so the fp2->table DMA chain is not stuck
behind Phase B on DVE.

out[b, du*9+dv -> r=dv*9+du, h0+hh, w] = core_out[w, hh*81 + (du*9+dv)].
"""

import numpy as np

import concourse.bass as bass
import concourse.bacc as bacc
import concourse.mybir as mybir
import concourse.tile as tile
from concourse import bass_utils

B_, C, H, W = 2, 48, 80, 128
R = 81
ND = 9
MD = 4
SW = W / (W - 1.0)
SH = H / (H - 1.0)
TH_X = 4.0 * (SW - 1.0)
TH_Y = 4.0 * (SH - 1.0)
NCORES = 8
NRB = 20            # output rows per core
PADL = 10
NCOLS = 11          # cols per table row
N_XS = 138          # x starts
Wp = 148            # padded width
NY = 40             # table y rows
NK = 11             # patch rows per pixel
ROWE = 640          # elems per table row (bf16): 528 used + pad (1280 B)
GE = NK * ROWE      # 7040 gather elems per pixel (14080 B, mult of 256)
NPX = 6016          # padded pixel rows in fp dram (40*148=5920 -> 47*128)
NTROW = NY * N_XS   # 5520 table rows
WSLOT = NRB * 8     # 160 idx slots per warp (wrapped 16p x 8 per rb)

F32 = mybir.dt.float32
I32 = mybir.dt.int32
I16 = mybir.dt.int16
BF16 = mybir.dt.bfloat16
AF = mybir.ActivationFunctionType
OP = mybir.AluOpType
LIN = np.linspace(-MD, MD, ND)

# ---- per-unit strategy tables (per warp, 9 units each) -----------------
# Entry = (act_taps, s_add, o_add):
#   act_taps: tap indices (0..2) whose mul runs on ACT; rest DVE ts.
#   s_add / o_add: 'D' (DVE tensor_tensor) or 'P' (Pool tensor_tensor).
# Adds merge across contiguous unit runs sharing an engine; Pool runs are
# chunked (POOL_CHUNK units) so they can start while ACT muls stream.
YW = [((), 'D', 'D')] * 9
XW0 = ([((0, 1, 2), 'P', 'P')] * 2 + [((0, 1, 2), 'D', 'P')] * 6
       + [((), 'D', 'P')] * 1)
XW1 = ([((0, 1, 2), 'P', 'P')] * 2 + [((0, 1, 2), 'D', 'P')] * 6
       + [((), 'D', 'P')] * 1)
XWS = (XW0, XW1)
# last row-block drains with no following work to overlap: run warp 1's
# x-pass on DVE so ACT (warp 0) and DVE (warp 1) drain in parallel
XWS_LAST = (XW0, [((), 'D', 'D')] * 9)
T1_POOL = False       # first dot-tree level on Pool
POOL_CHUNK_S = 4
POOL_CHUNK_O = 4
DOT_SPLIT = ((0, 9),)
PROD_SPLIT = ((0, 9),)


def mkap(t, dims, offset_elems=0):
    """Overlapping/custom AP on a dram tensor: dims = [[stride, count], ...]."""
    import bass_rust
    a = t.ap().copy() if hasattr(t, "ap") else t.copy()
    a.ap = bass_rust.VecI64Pair([list(d) for d in dims])
    if offset_elems:
        a.offset = a.offset + offset_elems
    return a


def _runs(strat, which):
    """Yield (start, end, engine) runs of equal add-engine assignment."""
    idx = 1 if which == 's' else 2
    runs = []
    s = 0
    for u in range(1, len(strat) + 1):
        if u == len(strat) or strat[u][idx] != strat[s][idx]:
            runs.append((s, u, strat[s][idx]))
            s = u
    return runs


def emit_pass(nc, t0, t1, strat, n, ins_fn, w_fn, out_ap_fn, t2=None):
    """Generic 3-tap pass over nu units.

    With t2: all three muls emitted up front (no cross-engine stall on the
    3rd mul), then s-add t0+=t1, o-add out=t0+t2.
    Without t2: t1 is reused for the 3rd mul after the s-add consumed it.
    """
    nu = len(strat)

    def mul(u, j, dst):
        xs = ins_fn(u)
        ws = w_fn(u)
        if j in strat[u][0]:
            nc.scalar.activation(dst, xs[j], AF.Copy, scale=ws[j])
        else:
            nc.vector.tensor_scalar(
                out=dst, in0=xs[j], scalar1=ws[j], scalar2=None, op0=OP.mult)

    def adds(which, tlast, dst_fn):
        for (u0, u1, e) in _runs(strat, which):
            pc = POOL_CHUNK_S if which == 's' else POOL_CHUNK_O
            step = pc if e == 'P' else (u1 - u0)
            for c0 in range(u0, u1, step):
                c1 = min(c0 + step, u1)
                eng = nc.vector if e == 'D' else nc.gpsimd
                sl = (slice(None), slice(c0, c1), slice(None))
                eng.tensor_tensor(out=dst_fn(c0, c1, sl), in0=t0[sl],
                                  in1=tlast[sl], op=OP.add)

    if t2 is not None:
        for u in range(nu):
            mul(u, 0, t0[:, u, :])
            mul(u, 1, t1[:, u, :])
            mul(u, 2, t2[:, u, :])
        adds('s', t1, lambda c0, c1, sl: t0[sl])
        adds('o', t2, lambda c0, c1, sl: out_ap_fn(c0, c1))
    else:
        for u in range(nu):
            mul(u, 0, t0[:, u, :])
            mul(u, 1, t1[:, u, :])
        adds('s', t1, lambda c0, c1, sl: t0[sl])
        for u in range(nu):
            mul(u, 2, t1[:, u, :])
        adds('o', t1, lambda c0, c1, sl: out_ap_fn(c0, c1))


def build_program():
    nc = bacc.Bacc(
        "TRN2",
        target_bir_lowering=False,
        debug=False,
        enable_asserts=False,
        num_devices=NCORES,
        num_swdge_queues=2,
    )

    f1s_d = nc.dram_tensor("f1s", [NPX, C], F32, kind="ExternalInput")
    f2s_d = nc.dram_tensor("f2s", [NPX, C], F32, kind="ExternalInput")
    # constants: [wio 1 | hcon 20 | y0con 1 | gx 18 | gy 18 | mgx 18 |
    # mgy 18 | bmx 20 | bmy 20] = 134 cols
    cst_d = nc.dram_tensor("cst", [128, 134], F32, kind="ExternalInput")

    fp1_d = nc.dram_tensor("fp1", [NPX, C], BF16, kind="Internal")
    fp2_d = nc.dram_tensor("fp2", [NPX, C], BF16, kind="Internal")
    tab1_d = nc.dram_tensor("tab1", [NTROW + 16, ROWE], BF16, kind="Internal")
    tab2_d = nc.dram_tensor("tab2", [NTROW + 16, ROWE], BF16, kind="Internal")
    iscr_d = nc.dram_tensor("iscr", [2, 16, WSLOT], I16, kind="Internal")
    out_d = nc.dram_tensor("out", [128, NRB * R], F32, kind="ExternalOutput")

    with tile.TileContext(nc) as tc:
        with tc.tile_pool(name="const", bufs=1) as constp:
            eps = constp.tile([128, 1], F32)
            nc.gpsimd.memset(eps[:], 1e-6)
            cst = constp.tile([128, 134], F32)
            nc.sync.dma_start(out=cst[:], in_=cst_d.ap())
            wio = cst[:, 0:1]
            hcon = cst[:, 1:21]
            y0con = cst[:, 21:22]
            gx = cst[:, 22:40]
            gy = cst[:, 40:58]
            mgx = cst[:, 58:76]
            mgy = cst[:, 76:94]
            bmx = cst[:, 94:114]
            bmy = cst[:, 114:134]

            # pools opened before norm so norm can close first (LIFO),
            # after Phase B: closing it right after Phase A emits a drain
            # that would stall Phase B on the table-build DMAs.
            fldcm = tc.tile_pool(name="fld", bufs=1)
            fldp = fldcm.__enter__()
            scrcm = tc.tile_pool(name="scr", bufs=1)
            scrp = scrcm.__enter__()
            normcm = tc.tile_pool(name="norm", bufs=1)
            normp = normcm.__enter__()

            # ------------ Phase A: normalize -> fp dram -> table ------------
            if True:
                lds = []
                for i, fsrc in enumerate((f2s_d, f1s_d)):
                    ld = normp.tile([128, 47, C], F32, tag=f"ld{i}",
                                    name=f"ld{i}")
                    src = mkap(fsrc, [[47 * C, 128], [1, 47 * C]])
                    nc.sync.dma_start(
                        out=ld[:].rearrange("p i c -> p (i c)"), in_=src)
                    lds.append(ld)
                for ld, fdst, tabd in ((lds[0], fp2_d, tab1_d),
                                       (lds[1], fp1_d, tab2_d)):
                    # norm on ACT+Pool so DVE stays free for Phase B and
                    # the table chain is not delayed behind it
                    sq = normp.tile([128, 47, C], F32, tag="sq")
                    nc.scalar.square(sq[:], ld[:])
                    ssq = normp.tile([128, 47], F32, tag="ssq")
                    nc.vector.tensor_reduce(
                        ssq[:], sq[:], axis=mybir.AxisListType.X, op=OP.add)
                    nc.scalar.activation(ssq[:], ssq[:], AF.Sqrt, bias=eps[:])
                    nc.vector.reciprocal(ssq[:], ssq[:])
                    nf = normp.tile([128, 47, C], BF16, tag="nf")
                    nc.gpsimd.tensor_mul(
                        nf[:], ld[:],
                        ssq[:].unsqueeze(-1).broadcast_to([128, 47, C]))
                    dst = mkap(fdst, [[47 * C, 128], [1, 47 * C]])
                    nc.sync.dma_start(
                        out=dst, in_=nf[:].rearrange("p i c -> p (i c)"))
                    # table build: tab[xw*NY + y] row = fp[y, xw..xw+10, :]
                    # (on the scalar-engine DMA queue so the next feature's
                    # load is not stuck behind it on the sync queue)
                    tsrc = mkap(fdst, [[C, N_XS], [Wp * C, NY],
                                       [1, NCOLS * C]])
                    tdst = mkap(tabd, [[NY * ROWE, N_XS], [ROWE, NY],
                                       [1, NCOLS * C]])
                    nc.scalar.dma_start(out=tdst, in_=tsrc)

            # ------------ Phase B: fields ----------------------------------
            # deprioritized so the scheduler prefers the norm->table chain
            # that gates the first gather
            _lowpri = tc.high_priority(offset=-1000000)
            _lowpri.__enter__()
            wA = []   # wA[warp][axis][tap] -> [128, NRB, ND] f32
            maskC = fldp.tile([128, NRB, R], BF16)

            for wi, sgn in ((0, 1.0), (1, -1.0)):
                vx = scrp.tile([128, NRB], F32, tag=f"vx{wi}", name=f"vx{wi}")
                nc.vector.tensor_scalar(
                    out=vx[:], in0=bmx, scalar1=sgn, scalar2=wio,
                    op0=OP.mult, op1=OP.add)
                nc.vector.tensor_scalar(
                    out=vx[:], in0=vx[:], scalar1=float(SW), scalar2=-0.5,
                    op0=OP.mult, op1=OP.add)
                vy = scrp.tile([128, NRB], F32, tag=f"vy{wi}", name=f"vy{wi}")
                nc.vector.tensor_scalar(
                    out=vy[:], in0=bmy, scalar1=sgn, scalar2=None,
                    op0=OP.mult)
                nc.vector.tensor_add(vy[:], vy[:], hcon)
                nc.vector.tensor_scalar(
                    out=vy[:], in0=vy[:], scalar1=float(SH), scalar2=-0.5,
                    op0=OP.mult, op1=OP.add)

                axes = []
                bases = []
                for ax, (v, th, gt) in enumerate(
                        ((vx, TH_X, gx), (vy, TH_Y, gy))):
                    pfx = f"w{wi}a{ax}"
                    t2_ = lambda tg: scrp.tile([128, NRB], F32,
                                               tag=pfx + tg, name=pfx + tg)
                    xi = scrp.tile([128, NRB], I32, tag=pfx + "i",
                                   name=pfx + "i")
                    nc.vector.tensor_copy(xi[:], v[:])
                    xf = t2_("xf")
                    nc.vector.tensor_copy(xf[:], xi[:])
                    er = t2_("er")
                    nc.vector.tensor_tensor(
                        out=er[:], in0=xf[:], in1=v[:], op=OP.is_gt)
                    base = t2_("b")
                    nc.vector.tensor_sub(base[:], xf[:], er[:])
                    fx = t2_("fx")
                    nc.vector.tensor_sub(fx[:], v[:], base[:])
                    sig = t2_("sg")
                    nc.vector.tensor_scalar(
                        out=sig[:], in0=fx[:], scalar1=float(th),
                        scalar2=None, op0=OP.is_lt)
                    t3_ = lambda tg: scrp.tile([128, NRB, ND], F32,
                                               tag=pfx + tg, name=pfx + tg)
                    gb = gt[:, wi * ND:(wi + 1) * ND]
                    gbb = gb.unsqueeze(1).broadcast_to([128, NRB, ND])
                    fxb = fx[:].unsqueeze(-1).broadcast_to([128, NRB, ND])
                    sgb = sig[:].unsqueeze(-1).broadcast_to([128, NRB, ND])
                    phi = t3_("ph")
                    nc.vector.tensor_tensor(
                        out=phi[:], in0=fxb, in1=gbb, op=OP.add)
                    thr = t2_("th")
                    nc.vector.tensor_scalar(
                        out=thr[:], in0=sig[:], scalar1=-1.0, scalar2=1.0,
                        op0=OP.mult, op1=OP.add)
                    ep = t3_("ep")
                    nc.vector.tensor_tensor(
                        out=ep[:], in0=phi[:],
                        in1=thr[:].unsqueeze(-1).broadcast_to([128, NRB, ND]),
                        op=OP.is_ge)
                    om = t3_("om")
                    nc.vector.tensor_sub(om[:], phi[:], ep[:])
                    nc.vector.tensor_tensor(
                        out=om[:], in0=om[:], in1=sgb, op=OP.add)
                    # A0 = (1-ep)(1-om), A1 = ep+om-2ep*om, A2 = ep*om
                    A2 = fldp.tile([128, NRB, ND], F32, tag=pfx + "A2",
                                   name=pfx + "A2")
                    nc.vector.tensor_mul(A2[:], ep[:], om[:])
                    s = t3_("s")
                    nc.vector.tensor_add(s[:], ep[:], om[:])
                    A1 = fldp.tile([128, NRB, ND], F32, tag=pfx + "A1",
                                   name=pfx + "A1")
                    nc.vector.scalar_tensor_tensor(
                        out=A1[:], in0=A2[:], scalar=-2.0, in1=s[:],
                        op0=OP.mult, op1=OP.add)
                    A0 = fldp.tile([128, NRB, ND], F32, tag=pfx + "A0",
                                   name=pfx + "A0")
                    nc.vector.scalar_tensor_tensor(
                        out=A0[:], in0=s[:], scalar=-1.0, in1=A2[:],
                        op0=OP.mult, op1=OP.add)
                    nc.vector.tensor_scalar(
                        out=A0[:], in0=A0[:], scalar1=1.0, scalar2=None,
                        op0=OP.add)
                    axes.append((A0, A1, A2))
                    bases.append((base, sig))
                wA.append(axes)

                # gather row index (one per pixel):
                # row = (sx+6)*40 + (sy-4-(h0-10)) = 40*sx + sy + (246-h0)
                (bx, sx), (by, sy) = bases[0], bases[1]
                sxf = scrp.tile([128, NRB], F32, tag=f"sx{wi}",
                                name=f"sx{wi}")
                nc.vector.tensor_sub(sxf[:], bx[:], sx[:])
                syf = scrp.tile([128, NRB], F32, tag=f"sy{wi}",
                                name=f"sy{wi}")
                nc.vector.tensor_sub(syf[:], by[:], sy[:])
                r0 = scrp.tile([128, NRB], F32, tag=f"r0{wi}", name=f"r0{wi}")
                nc.vector.tensor_scalar(
                    out=r0[:], in0=sxf[:], scalar1=float(NY),
                    scalar2=y0con, op0=OP.mult, op1=OP.add)
                nc.vector.tensor_tensor(
                    out=r0[:], in0=r0[:], in1=syf[:], op=OP.add)
                ix16 = scrp.tile([128, NRB], I16, tag=f"ix16{wi}",
                                 name=f"ix16{wi}")
                nc.vector.tensor_copy(ix16[:], r0[:])

                # wrap roundtrip: idx i = rb*128 + p  ->
                # iscr[wi][p%16, rb*8 + p//16]
                nc.sync.dma_start(
                    out=mkap(iscr_d, [[1, 8], [WSLOT, 16], [8, NRB]],
                             offset_elems=wi * 16 * WSLOT),
                    in_=ix16[:])

                # masks for this warp -> multiply into maskC (du-major)
                mx = scrp.tile([128, NRB, ND], F32, tag=f"mx{wi}",
                               name=f"mx{wi}")
                my = scrp.tile([128, NRB, ND], F32, tag=f"my{wi}",
                               name=f"my{wi}")
                for (mt, v, mgt, lim) in ((mx, vx, mgx, float(W)),
                                          (my, vy, mgy, float(H))):
                    pos = scrp.tile([128, NRB, ND], F32, tag=f"pos{wi}",
                                    name=f"pos{wi}{lim}")
                    nc.vector.tensor_tensor(
                        out=pos[:],
                        in0=v[:].unsqueeze(-1).broadcast_to([128, NRB, ND]),
                        in1=mgt[:, wi * ND:(wi + 1) * ND].unsqueeze(1)
                            .broadcast_to([128, NRB, ND]),
                        op=OP.add)
                    t = scrp.tile([128, NRB, ND], F32, tag=f"mt{wi}",
                                  name=f"mt{wi}{lim}")
                    nc.vector.tensor_scalar(
                        out=t[:], in0=pos[:], scalar1=-1.0, scalar2=lim,
                        op0=OP.mult, op1=OP.add)
                    nc.vector.tensor_scalar(
                        out=pos[:], in0=pos[:], scalar1=1.0, scalar2=None,
                        op0=OP.add)
                    nc.vector.tensor_tensor(
                        out=t[:], in0=t[:], in1=pos[:], op=OP.min)
                    nc.vector.tensor_scalar(
                        out=mt[:], in0=t[:], scalar1=0.0, scalar2=1.0,
                        op0=OP.max, op1=OP.min)
                mw = scrp.tile([128, NRB, ND, ND], F32, tag=f"mw{wi}",
                               name=f"mw{wi}")
                # du-major: mw[n, du, dv] = mx[du] * my[dv]
                nc.vector.tensor_tensor(
                    out=mw[:],
                    in0=mx[:].unsqueeze(-1).broadcast_to([128, NRB, ND, ND]),
                    in1=my[:].unsqueeze(2).broadcast_to([128, NRB, ND, ND]),
                    op=OP.mult)
                nc.vector.tensor_scalar(
                    out=mw[:], in0=mw[:], scalar1=0.999, scalar2=None,
                    op0=OP.is_ge)
                if wi == 0:
                    nc.vector.tensor_copy(
                        maskC[:], mw[:].rearrange("p n a b -> p n (a b)"))
                else:
                    nc.vector.tensor_mul(
                        maskC[:], maskC[:],
                        mw[:].rearrange("p n a b -> p n (a b)"))

            # read back wrapped idxs
            wrs = []
            for wi in range(2):
                wr = fldp.tile([128, WSLOT], I16, tag=f"wr{wi}",
                               name=f"wr{wi}")
                src = iscr_d.ap()[wi]
                nc.sync.dma_start(
                    out=wr[:],
                    in_=src.unsqueeze(0).broadcast_to([8, 16, WSLOT]))
                wrs.append(wr)

            _lowpri.__exit__(None, None, None)
            normcm.__exit__(None, None, None)
            scrcm.__exit__(None, None, None)

            # ------------ Phase C: per-rb loop ------------------------------
            tabs = (tab1_d, tab2_d)
            with (
                tc.tile_pool(name="oap", bufs=2) as oap,
                tc.tile_pool(name="pp", bufs=2) as pp,
                tc.tile_pool(name="typ", bufs=1) as typ,
                tc.tile_pool(name="txp", bufs=1) as txp,
                tc.tile_pool(name="pyp", bufs=2) as pyp,
                tc.tile_pool(name="fwp", bufs=2) as fwp,
                tc.tile_pool(name="dotp", bufs=1) as dotp,
            ):
                Ptiles = {}

                def emit_gathers(rb):
                    for wi in range(2):
                        P = pp.tile([128, 1, GE], BF16, tag=f"P{wi}",
                                    name=f"P{wi}_{rb}")
                        tv = mkap(tabs[wi], [[ROWE, NTROW], [1, GE]])
                        nc.gpsimd.dma_gather(
                            out_ap=P[:],
                            in_ap=tv,
                            idxs_ap=wrs[wi][:, rb * 8:(rb + 1) * 8],
                            num_idxs=128,
                            num_idxs_reg=128,
                            elem_size=GE,
                            elem_step=ROWE,
                            single_packet=False,
                            queue_num=wi,
                        )
                        Ptiles[(rb, wi)] = P

                pend = []

                def emit_dot(rb, fw):
                    # split along the x-unit axis at the Pool o-chunk
                    # boundary (units 0..3 | 4..8) so each piece can start
                    # as soon as its fw chunks land, instead of waiting for
                    # the whole x-pass.
                    prod = dotp.tile([128, R, C], BF16, tag="prod")
                    T1 = dotp.tile([128, R, C // 2], BF16, tag="T1")
                    T2 = dotp.tile([128, R, C // 4], BF16, tag="T2")
                    T3 = dotp.tile([128, R, C // 8], BF16, tag="T3")
                    T4 = dotp.tile([128, R, C // 16], BF16, tag="T4")
                    oacc = oap.tile([128, R], F32, tag="oacc")
                    for (c0, c1) in PROD_SPLIT:
                        r0, r1 = c0 * ND, c1 * ND
                        nc.vector.tensor_mul(
                            prod[:, r0:r1, :].rearrange("p r c -> p (r c)"),
                            fw[:, 0, c0:c1].rearrange(
                                "p u v c -> p (u v c)"),
                            fw[:, 1, c0:c1].rearrange(
                                "p u v c -> p (u v c)"))
                    for (c0, c1) in DOT_SPLIT:
                        r0, r1 = c0 * ND, c1 * ND
                        (nc.gpsimd if T1_POOL else nc.vector).tensor_tensor(
                            out=T1[:, r0:r1, :], in0=prod[:, r0:r1, 0:24],
                            in1=prod[:, r0:r1, 24:48], op=OP.add)
                        nc.gpsimd.tensor_tensor(
                            out=T2[:, r0:r1, :], in0=T1[:, r0:r1, 0:12],
                            in1=T1[:, r0:r1, 12:24], op=OP.add)
                        nc.gpsimd.tensor_tensor(
                            out=T3[:, r0:r1, :], in0=T2[:, r0:r1, 0:6],
                            in1=T2[:, r0:r1, 6:12], op=OP.add)
                        nc.gpsimd.tensor_tensor(
                            out=T4[:, r0:r1, :], in0=T3[:, r0:r1, 0:3],
                            in1=T3[:, r0:r1, 3:6], op=OP.add)
                        nc.vector.tensor_reduce(
                            oacc[:, r0:r1], T4[:, r0:r1, :],
                            axis=mybir.AxisListType.X, op=OP.add)
                        nc.vector.tensor_mul(
                            oacc[:, r0:r1], oacc[:, r0:r1],
                            maskC[:, rb, r0:r1])
                    nc.sync.dma_start(
                        out=mkap(out_d, [[NRB * R, 128], [1, R]],
                                 offset_elems=rb * R),
                        in_=oacc[:])

                emit_gathers(0)
                emit_gathers(1)
                for rb in range(NRB):
                    PF = Ptiles.pop((rb, 0))
                    PB = Ptiles.pop((rb, 1))
                    # ---- y-pass: per warp, 9 units -> Py[128, 2, 9, 528] --
                    Py = pyp.tile([128, 2, ND, NCOLS * C], BF16, tag="Py")
                    for wi in range(2):
                        P = (PF, PB)[wi]
                        ty0 = typ.tile([128, ND, NCOLS * C], BF16,
                                       tag="ty0")
                        ty1 = typ.tile([128, ND, NCOLS * C], BF16,
                                       tag="ty1")

                        def y_ins(idv, P=P, wi=wi):
                            q = idv if wi == 0 else (ND - 1 - idv)
                            return tuple(
                                P[:, 0, (q + j) * ROWE:
                                  (q + j) * ROWE + NCOLS * C]
                                for j in range(3))

                        def y_w(idv, rb=rb, wi=wi):
                            (A0y, A1y, A2y) = wA[wi][1]
                            return (A0y[:, rb, idv:idv + 1],
                                    A1y[:, rb, idv:idv + 1],
                                    A2y[:, rb, idv:idv + 1])

                        def y_out(u0, u1, Py=Py, wi=wi):
                            return Py[:, wi, u0:u1, :]

                        emit_pass(nc, ty0, ty1, YW, NCOLS * C,
                                  y_ins, y_w, y_out)

                    # ---- x-pass, warp-stage interleaved ------------------
                    # stages: [j0/j1 muls F,B] [s-adds F,B] [j2 muls F,B]
                    # [o-adds F,B] so ACT's mul stream never waits on adds.
                    fw = fwp.tile([128, 2, ND, ND, C], BF16, tag="fw")
                    txs = []
                    for wi in range(2):
                        tx0 = txp.tile([128, ND, ND * C], BF16,
                                       tag=f"tx0w{wi}", name=f"tx0w{wi}")
                        tx1 = txp.tile([128, ND, ND * C], BF16,
                                       tag=f"tx1w{wi}", name=f"tx1w{wi}")
                        txs.append((tx0, tx1))

                    xws = (XWS if rb < NRB - 1 else XWS_LAST)

                    def x_mul(wi, idu, j, dst, rb=rb, Py=Py, xws=xws):
                        q = idu if wi == 0 else (ND - 1 - idu)
                        x = Py[:, wi, :, (q + j) * C:(q + j + 1) * C]
                        w = wA[wi][0][j][:, rb, idu:idu + 1]
                        if j in xws[wi][idu][0]:
                            nc.scalar.activation(dst, x, AF.Copy, scale=w)
                        else:
                            nc.vector.tensor_scalar(
                                out=dst, in0=x, scalar1=w, scalar2=None,
                                op0=OP.mult)

                    def x_adds(wi, which, fw=fw, xws=xws):
                        t0, t1 = txs[wi]
                        for (u0, u1, e) in _runs(xws[wi], which):
                            pc = (POOL_CHUNK_S if which == 's'
                                  else POOL_CHUNK_O)
                            step = pc if e == 'P' else (u1 - u0)
                            for c0 in range(u0, u1, step):
                                c1 = min(c0 + step, u1)
                                eng = (nc.vector if e == 'D'
                                       else nc.gpsimd)
                                sl = (slice(None), slice(c0, c1),
                                      slice(None))
                                if which == 's':
                                    out = t0[sl]
                                else:
                                    out = fw[:, wi, c0:c1, :, :].rearrange(
                                        "p u v c -> p u (v c)")
                                eng.tensor_tensor(out=out, in0=t0[sl],
                                                  in1=t1[sl], op=OP.add)

                    for wi in range(2):
                        for u in range(ND):
                            x_mul(wi, u, 0, txs[wi][0][:, u, :])
                            x_mul(wi, u, 1, txs[wi][1][:, u, :])
                    if len(pend) > 1:
                        emit_dot(*pend.pop(0))
                    for wi in range(2):
                        x_adds(wi, 's')
                    for wi in range(2):
                        for u in range(ND):
                            x_mul(wi, u, 2, txs[wi][1][:, u, :])
                    for wi in range(2):
                        x_adds(wi, 'o')

                    if rb + 2 < NRB:
                        emit_gathers(rb + 2)
                    pend.append((rb, fw))
                for args in pend:
                    emit_dot(*args)

            fldcm.__exit__(None, None, None)

    nc.compile()
    return nc


def make_in_maps(feature1, feature2, BM):
    f1 = np.asarray(feature1, dtype=np.float32)
    f2 = np.asarray(feature2, dtype=np.float32)
    bm = np.asarray(BM, dtype=np.float32)

    wio = np.arange(W, dtype=np.float32).reshape(128, 1)

    def padded_slice(f, b, h0):
        ys = np.clip(h0 - 10 + np.arange(NY), 0, H - 1)
        xs = np.clip(np.arange(Wp) - PADL, 0, W - 1)
        s = f[b][:, ys][:, :, xs]                 # [C, NY, Wp]
        s = np.ascontiguousarray(s.transpose(1, 2, 0)).reshape(NY * Wp, C)
        out = np.zeros((NPX, C), np.float32)
        out[:NY * Wp] = s
        return out

    mgx = np.zeros((128, 2 * ND), np.float32)
    mgy = np.zeros((128, 2 * ND), np.float32)
    gx = np.zeros((128, 2 * ND), np.float32)
    gy = np.zeros((128, 2 * ND), np.float32)
    d = LIN.astype(np.float64)
    for wi, sgn in ((0, 1.0), (1, -1.0)):
        gx[:, wi * ND:(wi + 1) * ND] = (sgn * d * (SW - 1.0)).astype(
            np.float32)[None, :]
        gy[:, wi * ND:(wi + 1) * ND] = (sgn * d * (SH - 1.0)).astype(
            np.float32)[None, :]
        mgx[:, wi * ND:(wi + 1) * ND] = (sgn * d * SW).astype(
            np.float32)[None, :]
        mgy[:, wi * ND:(wi + 1) * ND] = (sgn * d * SH).astype(
            np.float32)[None, :]

    in_maps = []
    for k in range(NCORES):
        b = k // 4
        h0 = 20 * (k % 4)
        hcon = np.broadcast_to(
            (h0 + np.arange(NRB)).astype(np.float32)[None, :],
            (128, NRB)).copy()
        # row = 40*sx + sy + (246 - h0)
        y0con = np.full((128, 1), np.float32(246 - h0), np.float32)
        cst = np.concatenate([
            wio, hcon, y0con, gx, gy, mgx, mgy,
            np.ascontiguousarray(bm[b, 0, h0:h0 + NRB, :].T),
            np.ascontiguousarray(bm[b, 1, h0:h0 + NRB, :].T),
        ], axis=1).astype(np.float32)
        assert cst.shape == (128, 134), cst.shape
        in_maps.append({
            "f1s": padded_slice(f1, b, h0),
            "f2s": padded_slice(f2, b, h0),
            "cst": np.ascontiguousarray(cst),
        })
    return in_maps


_NC_CACHE = {}


def get_program():
    if "nc" not in _NC_CACHE:
        _NC_CACHE["nc"] = build_program()
    return _NC_CACHE["nc"]


# permutation: our r' = du*9+dv  ->  reference r = dv*9+du
_PERM = np.array([(rp % ND) * ND + rp // ND for rp in range(R)])


def core_to_ref(co):
    """co [128(w), NRB, R'] -> [R, NRB, 128] in reference r order."""
    inv = np.empty(R, np.int64)
    inv[_PERM] = np.arange(R)
    return co.transpose(2, 1, 0)[inv]


def assemble_output(results):
    out = np.zeros((B_, R, H, W), np.float32)
    for k in range(NCORES):
        b = k // 4
        h0 = 20 * (k % 4)
        co = results[k]["out"].reshape(128, NRB, R)   # [w, hh, r']
        out[b, :, h0:h0 + NRB, :] = core_to_ref(co)
    return out


def kernel(feature1, feature2, BM):
    nc = get_program()
    in_maps = make_in_maps(feature1, feature2, BM)
    res = bass_utils.run_bass_kernel_spmd(
        nc, in_maps, core_ids=list(range(NCORES)))
    return assemble_output(res.results)


# revision 74
# speedup vs baseline: 1.0014x; 1.0014x over previous
"""Bass/TRN2 kernel for nn_BilateralCostVolume — patch-gather scheme v2.

Sharding: core k handles batch b = k//4, output rows h in [20*(k%4), +20).
Per core, per pixel, per warp (F: +displacement on f2n; B: -displacement on
f1n) gather an 11x11 patch from a DRAM table laid out [xwin][y] so the 11
patch rows per pixel are CONTIGUOUS (one gather descriptor per pixel,
elem_step=640, elem_size=7040).  All 81 displacements are then computed
on-chip with static 3-tap separable interpolation (carry folded into
per-pixel weights), channel dot, mask.

Engine split (tunable tables below): y-pass fully on DVE (tensor_scalar
muls run in 4x perf mode, tensor_tensor adds in 2x, merged across the 9
units of a warp), x-pass muls mostly on ACT with adds on Pool (chunked so
they start while the ACT mul stream runs; stage-interleaved across warps
so no engine waits on another's adds), dot tree split DVE/Pool.  The dot
of row-block rb-1 is emitted inside rb's body (software pipelining) and
gathers are prefetched two row-blocks ahead.  Norm squares run on ACT and
the normalize-multiply on Pool so the fp2->table DMA chain is not stuck
behind Phase B on DVE.

out[b, du*9+dv -> r=dv*9+du, h0+hh, w] = core_out[w, hh*81 + (du*9+dv)].
"""

import numpy as np

import concourse.bass as bass
import concourse.bacc as bacc
import concourse.mybir as mybir
import concourse.tile as tile
from concourse import bass_utils

B_, C, H, W = 2, 48, 80, 128
R = 81
ND = 9
MD = 4
SW = W / (W - 1.0)
SH = H / (H - 1.0)
TH_X = 4.0 * (SW - 1.0)
TH_Y = 4.0 * (SH - 1.0)
NCORES = 8
NRB = 20            # output rows per core
PADL = 10
NCOLS = 11          # cols per table row
N_XS = 138          # x starts
Wp = 148            # padded width
NY = 40             # table y rows
NK = 11             # patch rows per pixel
ROWE = 640          # elems per table row (bf16): 528 used + pad (1280 B)
GE = NK * ROWE      # 7040 gather elems per pixel (14080 B, mult of 256)
NPX = 6016          # padded pixel rows in fp dram (40*148=5920 -> 47*128)
NTROW = NY * N_XS   # 5520 table rows
WSLOT = NRB * 8     # 160 idx slots per warp (wrapped 16p x 8 per rb)

F32 = mybir.dt.float32
I32 = mybir.dt.int32
I16 = mybir.dt.int16
BF16 = mybir.dt.bfloat16
AF = mybir.ActivationFunctionType
OP = mybir.AluOpType
LIN = np.linspace(-MD, MD, ND)

# ---- per-unit strategy tables (per warp, 9 units each) -----------------
# Entry = (act_taps, s_add, o_add):
#   act_taps: tap indices (0..2) whose mul runs on ACT; rest DVE ts.
#   s_add / o_add: 'D' (DVE tensor_tensor) or 'P' (Pool tensor_tensor).
# Adds merge across contiguous unit runs sharing an engine; Pool runs are
# chunked (POOL_CHUNK units) so they can start while ACT muls stream.
YW = [((), 'D', 'D')] * 9
XW0 = ([((0, 1, 2), 'P', 'P')] * 2 + [((0, 1, 2), 'D', 'P')] * 6
       + [((), 'D', 'P')] * 1)
XW1 = ([((0, 1, 2), 'P', 'P')] * 2 + [((0, 1, 2), 'D', 'P')] * 6
       + [((), 'D', 'P')] * 1)
XWS = (XW0, XW1)
# last row-block drains with no following work to overlap: run warp 1's
# x-pass on DVE so ACT (warp 0) and DVE (warp 1) drain in parallel, and
# warp 0's unit-8 adds on DVE too (Pool's tail chunk otherwise straggles)
XWS_LAST = ([((0, 1, 2), 'P', 'P')] * 2 + [((0, 1, 2), 'D', 'P')] * 6
            + [((), 'D', 'D')] * 1,
            [((), 'D', 'D')] * 9)
T1_POOL = False       # first dot-tree level on Pool
POOL_CHUNK_S = 4
POOL_CHUNK_O = 4
DOT_SPLIT = ((0, 9),)
PROD_SPLIT = ((0, 9),)


def mkap(t, dims, offset_elems=0):
    """Overlapping/custom AP on a dram tensor: dims = [[stride, count], ...]."""
    import bass_rust
    a = t.ap().copy() if hasattr(t, "ap") else t.copy()
    a.ap = bass_rust.VecI64Pair([list(d) for d in dims])
    if offset_elems:
        a.offset = a.offset + offset_elems
    return a


def _runs(strat, which):
    """Yield (start, end, engine) runs of equal add-engine assignment."""
    idx = 1 if which == 's' else 2
    runs = []
    s = 0
    for u in range(1, len(strat) + 1):
        if u == len(strat) or strat[u][idx] != strat[s][idx]:
            runs.append((s, u, strat[s][idx]))
            s = u
    return runs


def emit_pass(nc, t0, t1, strat, n, ins_fn, w_fn, out_ap_fn, t2=None):
    """Generic 3-tap pass over nu units.

    With t2: all three muls emitted up front (no cross-engine stall on the
    3rd mul), then s-add t0+=t1, o-add out=t0+t2.
    Without t2: t1 is reused for the 3rd mul after the s-add consumed it.
    """
    nu = len(strat)

    def mul(u, j, dst):
        xs = ins_fn(u)
        ws = w_fn(u)
        if j in strat[u][0]:
            nc.scalar.activation(dst, xs[j], AF.Copy, scale=ws[j])
        else:
            nc.vector.tensor_scalar(
                out=dst, in0=xs[j], scalar1=ws[j], scalar2=None, op0=OP.mult)

    def adds(which, tlast, dst_fn):
        for (u0, u1, e) in _runs(strat, which):
            pc = POOL_CHUNK_S if which == 's' else POOL_CHUNK_O
            step = pc if e == 'P' else (u1 - u0)
            for c0 in range(u0, u1, step):
                c1 = min(c0 + step, u1)
                eng = nc.vector if e == 'D' else nc.gpsimd
                sl = (slice(None), slice(c0, c1), slice(None))
                eng.tensor_tensor(out=dst_fn(c0, c1, sl), in0=t0[sl],
                                  in1=tlast[sl], op=OP.add)

    if t2 is not None:
        for u in range(nu):
            mul(u, 0, t0[:, u, :])
            mul(u, 1, t1[:, u, :])
            mul(u, 2, t2[:, u, :])
        adds('s', t1, lambda c0, c1, sl: t0[sl])
        adds('o', t2, lambda c0, c1, sl: out_ap_fn(c0, c1))
    else:
        for u in range(nu):
            mul(u, 0, t0[:, u, :])
            mul(u, 1, t1[:, u, :])
        adds('s', t1, lambda c0, c1, sl: t0[sl])
        for u in range(nu):
            mul(u, 2, t1[:, u, :])
        adds('o', t1, lambda c0, c1, sl: out_ap_fn(c0, c1))


def build_program():
    nc = bacc.Bacc(
        "TRN2",
        target_bir_lowering=False,
        debug=False,
        enable_asserts=False,
        num_devices=NCORES,
        num_swdge_queues=2,
    )

    f1s_d = nc.dram_tensor("f1s", [NPX, C], F32, kind="ExternalInput")
    f2s_d = nc.dram_tensor("f2s", [NPX, C], F32, kind="ExternalInput")
    # constants: [wio 1 | hcon 20 | y0con 1 | gx 18 | gy 18 | mgx 18 |
    # mgy 18 | bmx 20 | bmy 20] = 134 cols
    cst_d = nc.dram_tensor("cst", [128, 134], F32, kind="ExternalInput")

    fp1_d = nc.dram_tensor("fp1", [NPX, C], BF16, kind="Internal")
    fp2_d = nc.dram_tensor("fp2", [NPX, C], BF16, kind="Internal")
    tab1_d = nc.dram_tensor("tab1", [NTROW + 16, ROWE], BF16, kind="Internal")
    tab2_d = nc.dram_tensor("tab2", [NTROW + 16, ROWE], BF16, kind="Internal")
    iscr_d = nc.dram_tensor("iscr", [2, 16, WSLOT], I16, kind="Internal")
    out_d = nc.dram_tensor("out", [128, NRB * R], F32, kind="ExternalOutput")

    with tile.TileContext(nc) as tc:
        with tc.tile_pool(name="const", bufs=1) as constp:
            eps = constp.tile([128, 1], F32)
            nc.gpsimd.memset(eps[:], 1e-6)
            cst = constp.tile([128, 134], F32)
            nc.sync.dma_start(out=cst[:], in_=cst_d.ap())
            wio = cst[:, 0:1]
            hcon = cst[:, 1:21]
            y0con = cst[:, 21:22]
            gx = cst[:, 22:40]
            gy = cst[:, 40:58]
            mgx = cst[:, 58:76]
            mgy = cst[:, 76:94]
            bmx = cst[:, 94:114]
            bmy = cst[:, 114:134]

            # pools opened before norm so norm can close first (LIFO),
            # after Phase B: closing it right after Phase A emits a drain
            # that would stall Phase B on the table-build DMAs.
            fldcm = tc.tile_pool(name="fld", bufs=1)
            fldp = fldcm.__enter__()
            scrcm = tc.tile_pool(name="scr", bufs=1)
            scrp = scrcm.__enter__()
            normcm = tc.tile_pool(name="norm", bufs=1)
            normp = normcm.__enter__()

            # ------------ Phase A: normalize -> fp dram -> table ------------
            if True:
                lds = []
                for i, fsrc in enumerate((f2s_d, f1s_d)):
                    ld = normp.tile([128, 47, C], F32, tag=f"ld{i}",
                                    name=f"ld{i}")
                    src = mkap(fsrc, [[47 * C, 128], [1, 47 * C]])
                    nc.sync.dma_start(
                        out=ld[:].rearrange("p i c -> p (i c)"), in_=src)
                    lds.append(ld)
                for ld, fdst, tabd in ((lds[0], fp2_d, tab1_d),
                                       (lds[1], fp1_d, tab2_d)):
                    # norm on ACT+Pool so DVE stays free for Phase B and
                    # the table chain is not delayed behind it
                    sq = normp.tile([128, 47, C], F32, tag="sq")
                    nc.scalar.square(sq[:], ld[:])
                    ssq = normp.tile([128, 47], F32, tag="ssq")
                    nc.vector.tensor_reduce(
                        ssq[:], sq[:], axis=mybir.AxisListType.X, op=OP.add)
                    nc.scalar.activation(ssq[:], ssq[:], AF.Sqrt, bias=eps[:])
                    nc.vector.reciprocal(ssq[:], ssq[:])
                    nf = normp.tile([128, 47, C], BF16, tag="nf")
                    nc.gpsimd.tensor_mul(
                        nf[:], ld[:],
                        ssq[:].unsqueeze(-1).broadcast_to([128, 47, C]))
                    dst = mkap(fdst, [[47 * C, 128], [1, 47 * C]])
                    nc.sync.dma_start(
                        out=dst, in_=nf[:].rearrange("p i c -> p (i c)"))
                    # table build: tab[xw*NY + y] row = fp[y, xw..xw+10, :]
                    # (on the scalar-engine DMA queue so the next feature's
                    # load is not stuck behind it on the sync queue)
                    tsrc = mkap(fdst, [[C, N_XS], [Wp * C, NY],
                                       [1, NCOLS * C]])
                    tdst = mkap(tabd, [[NY * ROWE, N_XS], [ROWE, NY],
                                       [1, NCOLS * C]])
                    nc.scalar.dma_start(out=tdst, in_=tsrc)

            # ------------ Phase B: fields ----------------------------------
            # deprioritized so the scheduler prefers the norm->table chain
            # that gates the first gather
            _lowpri = tc.high_priority(offset=-1000000)
            _lowpri.__enter__()
            wA = []   # wA[warp][axis][tap] -> [128, NRB, ND] f32
            maskC = fldp.tile([128, NRB, R], BF16)

            for wi, sgn in ((0, 1.0), (1, -1.0)):
                vx = scrp.tile([128, NRB], F32, tag=f"vx{wi}", name=f"vx{wi}")
                nc.vector.tensor_scalar(
                    out=vx[:], in0=bmx, scalar1=sgn, scalar2=wio,
                    op0=OP.mult, op1=OP.add)
                nc.vector.tensor_scalar(
                    out=vx[:], in0=vx[:], scalar1=float(SW), scalar2=-0.5,
                    op0=OP.mult, op1=OP.add)
                vy = scrp.tile([128, NRB], F32, tag=f"vy{wi}", name=f"vy{wi}")
                nc.vector.tensor_scalar(
                    out=vy[:], in0=bmy, scalar1=sgn, scalar2=None,
                    op0=OP.mult)
                nc.vector.tensor_add(vy[:], vy[:], hcon)
                nc.vector.tensor_scalar(
                    out=vy[:], in0=vy[:], scalar1=float(SH), scalar2=-0.5,
                    op0=OP.mult, op1=OP.add)

                axes = []
                bases = []
                for ax, (v, th, gt) in enumerate(
                        ((vx, TH_X, gx), (vy, TH_Y, gy))):
                    pfx = f"w{wi}a{ax}"
                    t2_ = lambda tg: scrp.tile([128, NRB], F32,
                                               tag=pfx + tg, name=pfx + tg)
                    xi = scrp.tile([128, NRB], I32, tag=pfx + "i",
                                   name=pfx + "i")
                    nc.vector.tensor_copy(xi[:], v[:])
                    xf = t2_("xf")
                    nc.vector.tensor_copy(xf[:], xi[:])
                    er = t2_("er")
                    nc.vector.tensor_tensor(
                        out=er[:], in0=xf[:], in1=v[:], op=OP.is_gt)
                    base = t2_("b")
                    nc.vector.tensor_sub(base[:], xf[:], er[:])
                    fx = t2_("fx")
                    nc.vector.tensor_sub(fx[:], v[:], base[:])
                    sig = t2_("sg")
                    nc.vector.tensor_scalar(
                        out=sig[:], in0=fx[:], scalar1=float(th),
                        scalar2=None, op0=OP.is_lt)
                    t3_ = lambda tg: scrp.tile([128, NRB, ND], F32,
                                               tag=pfx + tg, name=pfx + tg)
                    gb = gt[:, wi * ND:(wi + 1) * ND]
                    gbb = gb.unsqueeze(1).broadcast_to([128, NRB, ND])
                    fxb = fx[:].unsqueeze(-1).broadcast_to([128, NRB, ND])
                    sgb = sig[:].unsqueeze(-1).broadcast_to([128, NRB, ND])
                    phi = t3_("ph")
                    nc.vector.tensor_tensor(
                        out=phi[:], in0=fxb, in1=gbb, op=OP.add)
                    thr = t2_("th")
                    nc.vector.tensor_scalar(
                        out=thr[:], in0=sig[:], scalar1=-1.0, scalar2=1.0,
                        op0=OP.mult, op1=OP.add)
                    ep = t3_("ep")
                    nc.vector.tensor_tensor(
                        out=ep[:], in0=phi[:],
                        in1=thr[:].unsqueeze(-1).broadcast_to([128, NRB, ND]),
                        op=OP.is_ge)
                    om = t3_("om")
                    nc.vector.tensor_sub(om[:], phi[:], ep[:])
                    nc.vector.tensor_tensor(
                        out=om[:], in0=om[:], in1=sgb, op=OP.add)
                    # A0 = (1-ep)(1-om), A1 = ep+om-2ep*om, A2 = ep*om
                    A2 = fldp.tile([128, NRB, ND], F32, tag=pfx + "A2",
                                   name=pfx + "A2")
                    nc.vector.tensor_mul(A2[:], ep[:], om[:])
                    s = t3_("s")
                    nc.vector.tensor_add(s[:], ep[:], om[:])
                    A1 = fldp.tile([128, NRB, ND], F32, tag=pfx + "A1",
                                   name=pfx + "A1")
                    nc.vector.scalar_tensor_tensor(
                        out=A1[:], in0=A2[:], scalar=-2.0, in1=s[:],
                        op0=OP.mult, op1=OP.add)
                    A0 = fldp.tile([128, NRB, ND], F32, tag=pfx + "A0",
                                   name=pfx + "A0")
                    nc.vector.scalar_tensor_tensor(
                        out=A0[:], in0=s[:], scalar=-1.0, in1=A2[:],
                        op0=OP.mult, op1=OP.add)
                    nc.vector.tensor_scalar(
                        out=A0[:], in0=A0[:], scalar1=1.0, scalar2=None,
                        op0=OP.add)
                    axes.append((A0, A1, A2))
                    bases.append((base, sig))
                wA.append(axes)

                # gather row index (one per pixel):
                # row = (sx+6)*40 + (sy-4-(h0-10)) = 40*sx + sy + (246-h0)
                (bx, sx), (by, sy) = bases[0], bases[1]
                sxf = scrp.tile([128, NRB], F32, tag=f"sx{wi}",
                                name=f"sx{wi}")
                nc.vector.tensor_sub(sxf[:], bx[:], sx[:])
                syf = scrp.tile([128, NRB], F32, tag=f"sy{wi}",
                                name=f"sy{wi}")
                nc.vector.tensor_sub(syf[:], by[:], sy[:])
                r0 = scrp.tile([128, NRB], F32, tag=f"r0{wi}", name=f"r0{wi}")
                nc.vector.tensor_scalar(
                    out=r0[:], in0=sxf[:], scalar1=float(NY),
                    scalar2=y0con, op0=OP.mult, op1=OP.add)
                nc.vector.tensor_tensor(
                    out=r0[:], in0=r0[:], in1=syf[:], op=OP.add)
                ix16 = scrp.tile([128, NRB], I16, tag=f"ix16{wi}",
                                 name=f"ix16{wi}")
                nc.vector.tensor_copy(ix16[:], r0[:])

                # wrap roundtrip: idx i = rb*128 + p  ->
                # iscr[wi][p%16, rb*8 + p//16]
                nc.sync.dma_start(
                    out=mkap(iscr_d, [[1, 8], [WSLOT, 16], [8, NRB]],
                             offset_elems=wi * 16 * WSLOT),
                    in_=ix16[:])

                # masks for this warp -> multiply into maskC (du-major)
                mx = scrp.tile([128, NRB, ND], F32, tag=f"mx{wi}",
                               name=f"mx{wi}")
                my = scrp.tile([128, NRB, ND], F32, tag=f"my{wi}",
                               name=f"my{wi}")
                for (mt, v, mgt, lim) in ((mx, vx, mgx, float(W)),
                                          (my, vy, mgy, float(H))):
                    pos = scrp.tile([128, NRB, ND], F32, tag=f"pos{wi}",
                                    name=f"pos{wi}{lim}")
                    nc.vector.tensor_tensor(
                        out=pos[:],
                        in0=v[:].unsqueeze(-1).broadcast_to([128, NRB, ND]),
                        in1=mgt[:, wi * ND:(wi + 1) * ND].unsqueeze(1)
                            .broadcast_to([128, NRB, ND]),
                        op=OP.add)
                    t = scrp.tile([128, NRB, ND], F32, tag=f"mt{wi}",
                                  name=f"mt{wi}{lim}")
                    nc.vector.tensor_scalar(
                        out=t[:], in0=pos[:], scalar1=-1.0, scalar2=lim,
                        op0=OP.mult, op1=OP.add)
                    nc.vector.tensor_scalar(
                        out=pos[:], in0=pos[:], scalar1=1.0, scalar2=None,
                        op0=OP.add)
                    nc.vector.tensor_tensor(
                        out=t[:], in0=t[:], in1=pos[:], op=OP.min)
                    nc.vector.tensor_scalar(
                        out=mt[:], in0=t[:], scalar1=0.0, scalar2=1.0,
                        op0=OP.max, op1=OP.min)
                mw = scrp.tile([128, NRB, ND, ND], F32, tag=f"mw{wi}",
                               name=f"mw{wi}")
                # du-major: mw[n, du, dv] = mx[du] * my[dv]
                nc.vector.tensor_tensor(
                    out=mw[:],
                    in0=mx[:].unsqueeze(-1).broadcast_to([128, NRB, ND, ND]),
                    in1=my[:].unsqueeze(2).broadcast_to([128, NRB, ND, ND]),
                    op=OP.mult)
                nc.vector.tensor_scalar(
                    out=mw[:], in0=mw[:], scalar1=0.999, scalar2=None,
                    op0=OP.is_ge)
                if wi == 0:
                    nc.vector.tensor_copy(
                        maskC[:], mw[:].rearrange("p n a b -> p n (a b)"))
                else:
                    nc.vector.tensor_mul(
                        maskC[:], maskC[:],
                        mw[:].rearrange("p n a b -> p n (a b)"))

            # read back wrapped idxs
            wrs = []
            for wi in range(2):
                wr = fldp.tile([128, WSLOT], I16, tag=f"wr{wi}",
                               name=f"wr{wi}")
                src = iscr_d.ap()[wi]
                nc.sync.dma_start(
                    out=wr[:],
                    in_=src.unsqueeze(0).broadcast_to([8, 16, WSLOT]))
                wrs.append(wr)

            _lowpri.__exit__(None, None, None)
            normcm.__exit__(None, None, None)
            scrcm.__exit__(None, None, None)

            # ------------ Phase C: per-rb loop ------------------------------
            tabs = (tab1_d, tab2_d)
            with (
                tc.tile_pool(name="oap", bufs=2) as oap,
                tc.tile_pool(name="pp", bufs=2) as pp,
                tc.tile_pool(name="typ", bufs=1) as typ,
                tc.tile_pool(name="txp", bufs=1) as txp,
                tc.tile_pool(name="pyp", bufs=2) as pyp,
                tc.tile_pool(name="fwp", bufs=2) as fwp,
                tc.tile_pool(name="dotp", bufs=1) as dotp,
            ):
                Ptiles = {}

                def emit_gathers(rb):
                    for wi in range(2):
                        P = pp.tile([128, 1, GE], BF16, tag=f"P{wi}",
                                    name=f"P{wi}_{rb}")
                        tv = mkap(tabs[wi], [[ROWE, NTROW], [1, GE]])
                        nc.gpsimd.dma_gather(
                            out_ap=P[:],
                            in_ap=tv,
                            idxs_ap=wrs[wi][:, rb * 8:(rb + 1) * 8],
                            num_idxs=128,
                            num_idxs_reg=128,
                            elem_size=GE,
                            elem_step=ROWE,
                            single_packet=False,
                            queue_num=wi,
                        )
                        Ptiles[(rb, wi)] = P

                pend = []

                def emit_dot(rb, fw):
                    # split along the x-unit axis at the Pool o-chunk
                    # boundary (units 0..3 | 4..8) so each piece can start
                    # as soon as its fw chunks land, instead of waiting for
                    # the whole x-pass.
                    prod = dotp.tile([128, R, C], BF16, tag="prod")
                    T1 = dotp.tile([128, R, C // 2], BF16, tag="T1")
                    T2 = dotp.tile([128, R, C // 4], BF16, tag="T2")
                    T3 = dotp.tile([128, R, C // 8], BF16, tag="T3")
                    T4 = dotp.tile([128, R, C // 16], BF16, tag="T4")
                    oacc = oap.tile([128, R], F32, tag="oacc")
                    for (c0, c1) in PROD_SPLIT:
                        r0, r1 = c0 * ND, c1 * ND
                        nc.vector.tensor_mul(
                            prod[:, r0:r1, :].rearrange("p r c -> p (r c)"),
                            fw[:, 0, c0:c1].rearrange(
                                "p u v c -> p (u v c)"),
                            fw[:, 1, c0:c1].rearrange(
                                "p u v c -> p (u v c)"))
                    for (c0, c1) in DOT_SPLIT:
                        r0, r1 = c0 * ND, c1 * ND
                        (nc.gpsimd if T1_POOL else nc.vector).tensor_tensor(
                            out=T1[:, r0:r1, :], in0=prod[:, r0:r1, 0:24],
                            in1=prod[:, r0:r1, 24:48], op=OP.add)
                        nc.gpsimd.tensor_tensor(
                            out=T2[:, r0:r1, :], in0=T1[:, r0:r1, 0:12],
                            in1=T1[:, r0:r1, 12:24], op=OP.add)
                        nc.gpsimd.tensor_tensor(
                            out=T3[:, r0:r1, :], in0=T2[:, r0:r1, 0:6],
                            in1=T2[:, r0:r1, 6:12], op=OP.add)
                        nc.gpsimd.tensor_tensor(
                            out=T4[:, r0:r1, :], in0=T3[:, r0:r1, 0:3],
                            in1=T3[:, r0:r1, 3:6], op=OP.add)
                        nc.vector.tensor_reduce(
                            oacc[:, r0:r1], T4[:, r0:r1, :],
                            axis=mybir.AxisListType.X, op=OP.add)
                        nc.vector.tensor_mul(
                            oacc[:, r0:r1], oacc[:, r0:r1],
                            maskC[:, rb, r0:r1])
                    nc.sync.dma_start(
                        out=mkap(out_d, [[NRB * R, 128], [1, R]],
                                 offset_elems=rb * R),
                        in_=oacc[:])

                emit_gathers(0)
                emit_gathers(1)
                for rb in range(NRB):
                    PF = Ptiles.pop((rb, 0))
                    PB = Ptiles.pop((rb, 1))
                    # ---- y-pass: per warp, 9 units -> Py[128, 2, 9, 528] --
                    Py = pyp.tile([128, 2, ND, NCOLS * C], BF16, tag="Py")
                    for wi in range(2):
                        P = (PF, PB)[wi]
                        ty0 = typ.tile([128, ND, NCOLS * C], BF16,
                                       tag="ty0")
                        ty1 = typ.tile([128, ND, NCOLS * C], BF16,
                                       tag="ty1")

                        def y_ins(idv, P=P, wi=wi):
                            q = idv if wi == 0 else (ND - 1 - idv)
                            return tuple(
                                P[:, 0, (q + j) * ROWE:
                                  (q + j) * ROWE + NCOLS * C]
                                for j in range(3))

                        def y_w(idv, rb=rb, wi=wi):
                            (A0y, A1y, A2y) = wA[wi][1]
                            return (A0y[:, rb, idv:idv + 1],
                                    A1y[:, rb, idv:idv + 1],
                                    A2y[:, rb, idv:idv + 1])

                        def y_out(u0, u1, Py=Py, wi=wi):
                            return Py[:, wi, u0:u1, :]

                        emit_pass(nc, ty0, ty1, YW, NCOLS * C,
                                  y_ins, y_w, y_out)

                    # ---- x-pass, warp-stage interleaved ------------------
                    # stages: [j0/j1 muls F,B] [s-adds F,B] [j2 muls F,B]
                    # [o-adds F,B] so ACT's mul stream never waits on adds.
                    fw = fwp.tile([128, 2, ND, ND, C], BF16, tag="fw")
                    txs = []
                    for wi in range(2):
                        tx0 = txp.tile([128, ND, ND * C], BF16,
                                       tag=f"tx0w{wi}", name=f"tx0w{wi}")
                        tx1 = txp.tile([128, ND, ND * C], BF16,
                                       tag=f"tx1w{wi}", name=f"tx1w{wi}")
                        txs.append((tx0, tx1))

                    xws = (XWS if rb < NRB - 1 else XWS_LAST)

                    def x_mul(wi, idu, j, dst, rb=rb, Py=Py, xws=xws):
                        q = idu if wi == 0 else (ND - 1 - idu)
                        x = Py[:, wi, :, (q + j) * C:(q + j + 1) * C]
                        w = wA[wi][0][j][:, rb, idu:idu + 1]
                        if j in xws[wi][idu][0]:
                            nc.scalar.activation(dst, x, AF.Copy, scale=w)
                        else:
                            nc.vector.tensor_scalar(
                                out=dst, in0=x, scalar1=w, scalar2=None,
                                op0=OP.mult)

                    def x_adds(wi, which, fw=fw, xws=xws):
                        t0, t1 = txs[wi]
                        for (u0, u1, e) in _runs(xws[wi], which):
                            pc = (POOL_CHUNK_S if which == 's'
                                  else POOL_CHUNK_O)
                            step = pc if e == 'P' else (u1 - u0)
                            for c0 in range(u0, u1, step):
                                c1 = min(c0 + step, u1)
                                eng = (nc.vector if e == 'D'
                                       else nc.gpsimd)
                                sl = (slice(None), slice(c0, c1),
                                      slice(None))
                                if which == 's':
                                    out = t0[sl]
                                else:
                                    out = fw[:, wi, c0:c1, :, :].rearrange(
                                        "p u v c -> p u (v c)")
                                eng.tensor_tensor(out=out, in0=t0[sl],
                                                  in1=t1[sl], op=OP.add)

                    for wi in range(2):
                        for u in range(ND):
                            x_mul(wi, u, 0, txs[wi][0][:, u, :])
                            x_mul(wi, u, 1, txs[wi][1][:, u, :])
                    if len(pend) > 1:
                        emit_dot(*pend.pop(0))
                    for wi in range(2):
                        x_adds(wi, 's')
                    for wi in range(2):
                        for u in range(ND):
                            x_mul(wi, u, 2, txs[wi][1][:, u, :])
                    for wi in range(2):
                        x_adds(wi, 'o')

                    if rb + 2 < NRB:
                        emit_gathers(rb + 2)
                    pend.append((rb, fw))
                for args in pend:
                    emit_dot(*args)

            fldcm.__exit__(None, None, None)

    nc.compile()
    return nc


def make_in_maps(feature1, feature2, BM):
    f1 = np.asarray(feature1, dtype=np.float32)
    f2 = np.asarray(feature2, dtype=np.float32)
    bm = np.asarray(BM, dtype=np.float32)

    wio = np.arange(W, dtype=np.float32).reshape(128, 1)

    def padded_slice(f, b, h0):
        ys = np.clip(h0 - 10 + np.arange(NY), 0, H - 1)
        xs = np.clip(np.arange(Wp) - PADL, 0, W - 1)
        s = f[b][:, ys][:, :, xs]                 # [C, NY, Wp]
        s = np.ascontiguousarray(s.transpose(1, 2, 0)).reshape(NY * Wp, C)
        out = np.zeros((NPX, C), np.float32)
        out[:NY * Wp] = s
        return out

    mgx = np.zeros((128, 2 * ND), np.float32)
    mgy = np.zeros((128, 2 * ND), np.float32)
    gx = np.zeros((128, 2 * ND), np.float32)
    gy = np.zeros((128, 2 * ND), np.float32)
    d = LIN.astype(np.float64)
    for wi, sgn in ((0, 1.0), (1, -1.0)):
        gx[:, wi * ND:(wi + 1) * ND] = (sgn * d * (SW - 1.0)).astype(
            np.float32)[None, :]
        gy[:, wi * ND:(wi + 1) * ND] = (sgn * d * (SH - 1.0)).astype(
            np.float32)[None, :]
        mgx[:, wi * ND:(wi + 1) * ND] = (sgn * d * SW).astype(
            np.float32)[None, :]
        mgy[:, wi * ND:(wi + 1) * ND] = (sgn * d * SH).astype(
            np.float32)[None, :]

    in_maps = []
    for k in range(NCORES):
        b = k // 4
        h0 = 20 * (k % 4)
        hcon = np.broadcast_to(
            (h0 + np.arange(NRB)).astype(np.float32)[None, :],
            (128, NRB)).copy()
        # row = 40*sx + sy + (246 - h0)
        y0con = np.full((128, 1), np.float32(246 - h0), np.float32)
        cst = np.concatenate([
            wio, hcon, y0con, gx, gy, mgx, mgy,
            np.ascontiguousarray(bm[b, 0, h0:h0 + NRB, :].T),
            np.ascontiguousarray(bm[b, 1, h0:h0 + NRB, :].T),
        ], axis=1).astype(np.float32)
        assert cst.shape == (128, 134), cst.shape
        in_maps.append({
            "f1s": padded_slice(f1, b, h0),
            "f2s": padded_slice(f2, b, h0),
            "cst": np.ascontiguousarray(cst),
        })
    return in_maps


_NC_CACHE = {}


def get_program():
    if "nc" not in _NC_CACHE:
        _NC_CACHE["nc"] = build_program()
    return _NC_CACHE["nc"]


# permutation: our r' = du*9+dv  ->  reference r = dv*9+du
_PERM = np.array([(rp % ND) * ND + rp // ND for rp in range(R)])


def core_to_ref(co):
    """co [128(w), NRB, R'] -> [R, NRB, 128] in reference r order."""
    inv = np.empty(R, np.int64)
    inv[_PERM] = np.arange(R)
    return co.transpose(2, 1, 0)[inv]


def assemble_output(results):
    out = np.zeros((B_, R, H, W), np.float32)
    for k in range(NCORES):
        b = k // 4
        h0 = 20 * (k % 4)
        co = results[k]["out"].reshape(128, NRB, R)   # [w, hh, r']
        out[b, :, h0:h0 + NRB, :] = core_to_ref(co)
    return out


def kernel(feature1, feature2, BM):
    nc = get_program()
    in_maps = make_in_maps(feature1, feature2, BM)
    res = bass_utils.run_bass_kernel_spmd(
        nc, in_maps, core_ids=list(range(NCORES)))
    return assemble_output(res.results)


# revision 78
# speedup vs baseline: 1.0051x; 1.0037x over previous
"""Bass/TRN2 kernel for nn_BilateralCostVolume — patch-gather scheme v2.

Sharding: core k handles batch b = k//4, output rows h in [20*(k%4), +20).
Per core, per pixel, per warp (F: +displacement on f2n; B: -displacement on
f1n) gather an 11x11 patch from a DRAM table laid out [xwin][y] so the 11
patch rows per pixel are CONTIGUOUS (one gather descriptor per pixel,
elem_step=640, elem_size=7040).  All 81 displacements are then computed
on-chip with static 3-tap separable interpolation (carry folded into
per-pixel weights), channel dot, mask.

Engine split (tunable tables below): y-pass fully on DVE (tensor_scalar
muls run in 4x perf mode, tensor_tensor adds in 2x, merged across the 9
units of a warp), x-pass muls mostly on ACT with adds on Pool (chunked so
they start while the ACT mul stream runs; stage-interleaved across warps
so no engine waits on another's adds), dot tree split DVE/Pool.  The dot
of row-block rb-1 is emitted inside rb's body (software pipelining) and
gathers are prefetched two row-blocks ahead.  Norm squares run on ACT and
the normalize-multiply on Pool so the fp2->table DMA chain is not stuck
behind Phase B on DVE.

out[b, du*9+dv -> r=dv*9+du, h0+hh, w] = core_out[w, hh*81 + (du*9+dv)].
"""

import numpy as np

import concourse.bass as bass
import concourse.bacc as bacc
import concourse.mybir as mybir
import concourse.tile as tile
from concourse import bass_utils

B_, C, H, W = 2, 48, 80, 128
R = 81
ND = 9
MD = 4
SW = W / (W - 1.0)
SH = H / (H - 1.0)
TH_X = 4.0 * (SW - 1.0)
TH_Y = 4.0 * (SH - 1.0)
NCORES = 8
NRB = 20            # output rows per core
PADL = 10
NCOLS = 11          # cols per table row
N_XS = 138          # x starts
Wp = 148            # padded width
NY = 40             # table y rows
NK = 11             # patch rows per pixel
ROWE = 640          # elems per table row (bf16): 528 used + pad (1280 B)
GE = NK * ROWE      # 7040 gather elems per pixel (14080 B, mult of 256)
NPX = 6016          # padded pixel rows in fp dram (40*148=5920 -> 47*128)
NTROW = NY * N_XS   # 5520 table rows
WSLOT = NRB * 8     # 160 idx slots per warp (wrapped 16p x 8 per rb)

F32 = mybir.dt.float32
I32 = mybir.dt.int32
I16 = mybir.dt.int16
BF16 = mybir.dt.bfloat16
AF = mybir.ActivationFunctionType
OP = mybir.AluOpType
LIN = np.linspace(-MD, MD, ND)

# ---- per-unit strategy tables (per warp, 9 units each) -----------------
# Entry = (act_taps, s_add, o_add):
#   act_taps: tap indices (0..2) whose mul runs on ACT; rest DVE ts.
#   s_add / o_add: 'D' (DVE tensor_tensor) or 'P' (Pool tensor_tensor).
# Adds merge across contiguous unit runs sharing an engine; Pool runs are
# chunked (POOL_CHUNK units) so they can start while ACT muls stream.
YW = [((), 'D', 'D')] * 9
XW0 = ([((0, 1, 2), 'P', 'P')] * 2 + [((0, 1, 2), 'D', 'P')] * 6
       + [((), 'D', 'P')] * 1)
XW1 = ([((0, 1, 2), 'P', 'P')] * 2 + [((0, 1, 2), 'D', 'P')] * 6
       + [((), 'D', 'P')] * 1)
XWS = (XW0, XW1)
# last row-block drains with no following work to overlap: run warp 1's
# x-pass on DVE so ACT (warp 0) and DVE (warp 1) drain in parallel, and
# warp 0's unit-8 adds on DVE too (Pool's tail chunk otherwise straggles)
XWS_LAST = ([((0, 1, 2), 'P', 'P')] * 2 + [((0, 1, 2), 'D', 'P')] * 6
            + [((), 'D', 'D')] * 1,
            [((), 'D', 'D')] * 9)
# late row-blocks: unit-8's adds (Pool's 1-unit straggler chunk) on DVE
XWS_U8D = ([((0, 1, 2), 'P', 'P')] * 2 + [((0, 1, 2), 'D', 'P')] * 6
           + [((), 'D', 'D')] * 1,
           [((0, 1, 2), 'P', 'P')] * 2 + [((0, 1, 2), 'D', 'P')] * 6
           + [((), 'D', 'D')] * 1)
XWS_TAB = {NRB - 1: XWS_LAST,
           NRB - 2: XWS_U8D, NRB - 3: XWS_U8D, NRB - 4: XWS_U8D}
T1_POOL = False       # first dot-tree level on Pool
POOL_CHUNK_S = 4
POOL_CHUNK_O = 4
DOT_SPLIT = ((0, 9),)
PROD_SPLIT = ((0, 9),)


def mkap(t, dims, offset_elems=0):
    """Overlapping/custom AP on a dram tensor: dims = [[stride, count], ...]."""
    import bass_rust
    a = t.ap().copy() if hasattr(t, "ap") else t.copy()
    a.ap = bass_rust.VecI64Pair([list(d) for d in dims])
    if offset_elems:
        a.offset = a.offset + offset_elems
    return a


def _runs(strat, which):
    """Yield (start, end, engine) runs of equal add-engine assignment."""
    idx = 1 if which == 's' else 2
    runs = []
    s = 0
    for u in range(1, len(strat) + 1):
        if u == len(strat) or strat[u][idx] != strat[s][idx]:
            runs.append((s, u, strat[s][idx]))
            s = u
    return runs


def emit_pass(nc, t0, t1, strat, n, ins_fn, w_fn, out_ap_fn, t2=None):
    """Generic 3-tap pass over nu units.

    With t2: all three muls emitted up front (no cross-engine stall on the
    3rd mul), then s-add t0+=t1, o-add out=t0+t2.
    Without t2: t1 is reused for the 3rd mul after the s-add consumed it.
    """
    nu = len(strat)

    def mul(u, j, dst):
        xs = ins_fn(u)
        ws = w_fn(u)
        if j in strat[u][0]:
            nc.scalar.activation(dst, xs[j], AF.Copy, scale=ws[j])
        else:
            nc.vector.tensor_scalar(
                out=dst, in0=xs[j], scalar1=ws[j], scalar2=None, op0=OP.mult)

    def adds(which, tlast, dst_fn):
        for (u0, u1, e) in _runs(strat, which):
            pc = POOL_CHUNK_S if which == 's' else POOL_CHUNK_O
            step = pc if e == 'P' else (u1 - u0)
            for c0 in range(u0, u1, step):
                c1 = min(c0 + step, u1)
                eng = nc.vector if e == 'D' else nc.gpsimd
                sl = (slice(None), slice(c0, c1), slice(None))
                eng.tensor_tensor(out=dst_fn(c0, c1, sl), in0=t0[sl],
                                  in1=tlast[sl], op=OP.add)

    if t2 is not None:
        for u in range(nu):
            mul(u, 0, t0[:, u, :])
            mul(u, 1, t1[:, u, :])
            mul(u, 2, t2[:, u, :])
        adds('s', t1, lambda c0, c1, sl: t0[sl])
        adds('o', t2, lambda c0, c1, sl: out_ap_fn(c0, c1))
    else:
        for u in range(nu):
            mul(u, 0, t0[:, u, :])
            mul(u, 1, t1[:, u, :])
        adds('s', t1, lambda c0, c1, sl: t0[sl])
        for u in range(nu):
            mul(u, 2, t1[:, u, :])
        adds('o', t1, lambda c0, c1, sl: out_ap_fn(c0, c1))


def build_program():
    nc = bacc.Bacc(
        "TRN2",
        target_bir_lowering=False,
        debug=False,
        enable_asserts=False,
        num_devices=NCORES,
        num_swdge_queues=2,
    )

    f1s_d = nc.dram_tensor("f1s", [NPX, C], F32, kind="ExternalInput")
    f2s_d = nc.dram_tensor("f2s", [NPX, C], F32, kind="ExternalInput")
    # constants: [wio 1 | hcon 20 | y0con 1 | gx 18 | gy 18 | mgx 18 |
    # mgy 18 | bmx 20 | bmy 20] = 134 cols
    cst_d = nc.dram_tensor("cst", [128, 134], F32, kind="ExternalInput")

    fp1_d = nc.dram_tensor("fp1", [NPX, C], BF16, kind="Internal")
    fp2_d = nc.dram_tensor("fp2", [NPX, C], BF16, kind="Internal")
    tab1_d = nc.dram_tensor("tab1", [NTROW + 16, ROWE], BF16, kind="Internal")
    tab2_d = nc.dram_tensor("tab2", [NTROW + 16, ROWE], BF16, kind="Internal")
    iscr_d = nc.dram_tensor("iscr", [2, 16, WSLOT], I16, kind="Internal")
    out_d = nc.dram_tensor("out", [128, NRB * R], F32, kind="ExternalOutput")

    with tile.TileContext(nc) as tc:
        with tc.tile_pool(name="const", bufs=1) as constp:
            eps = constp.tile([128, 1], F32)
            nc.gpsimd.memset(eps[:], 1e-6)
            cst = constp.tile([128, 134], F32)
            nc.sync.dma_start(out=cst[:], in_=cst_d.ap())
            wio = cst[:, 0:1]
            hcon = cst[:, 1:21]
            y0con = cst[:, 21:22]
            gx = cst[:, 22:40]
            gy = cst[:, 40:58]
            mgx = cst[:, 58:76]
            mgy = cst[:, 76:94]
            bmx = cst[:, 94:114]
            bmy = cst[:, 114:134]

            # pools opened before norm so norm can close first (LIFO),
            # after Phase B: closing it right after Phase A emits a drain
            # that would stall Phase B on the table-build DMAs.
            fldcm = tc.tile_pool(name="fld", bufs=1)
            fldp = fldcm.__enter__()
            scrcm = tc.tile_pool(name="scr", bufs=1)
            scrp = scrcm.__enter__()
            normcm = tc.tile_pool(name="norm", bufs=1)
            normp = normcm.__enter__()

            # ------------ Phase A: normalize -> fp dram -> table ------------
            if True:
                lds = []
                for i, fsrc in enumerate((f2s_d, f1s_d)):
                    ld = normp.tile([128, 47, C], F32, tag=f"ld{i}",
                                    name=f"ld{i}")
                    src = mkap(fsrc, [[47 * C, 128], [1, 47 * C]])
                    nc.sync.dma_start(
                        out=ld[:].rearrange("p i c -> p (i c)"), in_=src)
                    lds.append(ld)
                for ld, fdst, tabd in ((lds[0], fp2_d, tab1_d),
                                       (lds[1], fp1_d, tab2_d)):
                    # norm on ACT+Pool so DVE stays free for Phase B and
                    # the table chain is not delayed behind it
                    sq = normp.tile([128, 47, C], F32, tag="sq")
                    nc.scalar.square(sq[:], ld[:])
                    ssq = normp.tile([128, 47], F32, tag="ssq")
                    nc.vector.tensor_reduce(
                        ssq[:], sq[:], axis=mybir.AxisListType.X, op=OP.add)
                    nc.scalar.activation(ssq[:], ssq[:], AF.Sqrt, bias=eps[:])
                    nc.vector.reciprocal(ssq[:], ssq[:])
                    nf = normp.tile([128, 47, C], BF16, tag="nf")
                    nc.gpsimd.tensor_mul(
                        nf[:], ld[:],
                        ssq[:].unsqueeze(-1).broadcast_to([128, 47, C]))
                    dst = mkap(fdst, [[47 * C, 128], [1, 47 * C]])
                    nc.sync.dma_start(
                        out=dst, in_=nf[:].rearrange("p i c -> p (i c)"))
                    # table build: tab[xw*NY + y] row = fp[y, xw..xw+10, :]
                    # (on the scalar-engine DMA queue so the next feature's
                    # load is not stuck behind it on the sync queue)
                    tsrc = mkap(fdst, [[C, N_XS], [Wp * C, NY],
                                       [1, NCOLS * C]])
                    tdst = mkap(tabd, [[NY * ROWE, N_XS], [ROWE, NY],
                                       [1, NCOLS * C]])
                    nc.scalar.dma_start(out=tdst, in_=tsrc)

            # ------------ Phase B: fields ----------------------------------
            # deprioritized so the scheduler prefers the norm->table chain
            # that gates the first gather
            _lowpri = tc.high_priority(offset=-1000000)
            _lowpri.__enter__()
            wA = []   # wA[warp][axis][tap] -> [128, NRB, ND] f32
            maskC = fldp.tile([128, NRB, R], BF16)

            for wi, sgn in ((0, 1.0), (1, -1.0)):
                vx = scrp.tile([128, NRB], F32, tag=f"vx{wi}", name=f"vx{wi}")
                nc.vector.tensor_scalar(
                    out=vx[:], in0=bmx, scalar1=sgn, scalar2=wio,
                    op0=OP.mult, op1=OP.add)
                nc.vector.tensor_scalar(
                    out=vx[:], in0=vx[:], scalar1=float(SW), scalar2=-0.5,
                    op0=OP.mult, op1=OP.add)
                vy = scrp.tile([128, NRB], F32, tag=f"vy{wi}", name=f"vy{wi}")
                nc.vector.tensor_scalar(
                    out=vy[:], in0=bmy, scalar1=sgn, scalar2=None,
                    op0=OP.mult)
                nc.vector.tensor_add(vy[:], vy[:], hcon)
                nc.vector.tensor_scalar(
                    out=vy[:], in0=vy[:], scalar1=float(SH), scalar2=-0.5,
                    op0=OP.mult, op1=OP.add)

                axes = []
                bases = []
                for ax, (v, th, gt) in enumerate(
                        ((vx, TH_X, gx), (vy, TH_Y, gy))):
                    pfx = f"w{wi}a{ax}"
                    t2_ = lambda tg: scrp.tile([128, NRB], F32,
                                               tag=pfx + tg, name=pfx + tg)
                    xi = scrp.tile([128, NRB], I32, tag=pfx + "i",
                                   name=pfx + "i")
                    nc.vector.tensor_copy(xi[:], v[:])
                    xf = t2_("xf")
                    nc.vector.tensor_copy(xf[:], xi[:])
                    er = t2_("er")
                    nc.vector.tensor_tensor(
                        out=er[:], in0=xf[:], in1=v[:], op=OP.is_gt)
                    base = t2_("b")
                    nc.vector.tensor_sub(base[:], xf[:], er[:])
                    fx = t2_("fx")
                    nc.vector.tensor_sub(fx[:], v[:], base[:])
                    sig = t2_("sg")
                    nc.vector.tensor_scalar(
                        out=sig[:], in0=fx[:], scalar1=float(th),
                        scalar2=None, op0=OP.is_lt)
                    t3_ = lambda tg: scrp.tile([128, NRB, ND], F32,
                                               tag=pfx + tg, name=pfx + tg)
                    gb = gt[:, wi * ND:(wi + 1) * ND]
                    gbb = gb.unsqueeze(1).broadcast_to([128, NRB, ND])
                    fxb = fx[:].unsqueeze(-1).broadcast_to([128, NRB, ND])
                    sgb = sig[:].unsqueeze(-1).broadcast_to([128, NRB, ND])
                    phi = t3_("ph")
                    nc.vector.tensor_tensor(
                        out=phi[:], in0=fxb, in1=gbb, op=OP.add)
                    thr = t2_("th")
                    nc.vector.tensor_scalar(
                        out=thr[:], in0=sig[:], scalar1=-1.0, scalar2=1.0,
                        op0=OP.mult, op1=OP.add)
                    ep = t3_("ep")
                    nc.vector.tensor_tensor(
                        out=ep[:], in0=phi[:],
                        in1=thr[:].unsqueeze(-1).broadcast_to([128, NRB, ND]),
                        op=OP.is_ge)
                    om = t3_("om")
                    nc.vector.tensor_sub(om[:], phi[:], ep[:])
                    nc.vector.tensor_tensor(
                        out=om[:], in0=om[:], in1=sgb, op=OP.add)
                    # A0 = (1-ep)(1-om), A1 = ep+om-2ep*om, A2 = ep*om
                    A2 = fldp.tile([128, NRB, ND], F32, tag=pfx + "A2",
                                   name=pfx + "A2")
                    nc.vector.tensor_mul(A2[:], ep[:], om[:])
                    s = t3_("s")
                    nc.vector.tensor_add(s[:], ep[:], om[:])
                    A1 = fldp.tile([128, NRB, ND], F32, tag=pfx + "A1",
                                   name=pfx + "A1")
                    nc.vector.scalar_tensor_tensor(
                        out=A1[:], in0=A2[:], scalar=-2.0, in1=s[:],
                        op0=OP.mult, op1=OP.add)
                    A0 = fldp.tile([128, NRB, ND], F32, tag=pfx + "A0",
                                   name=pfx + "A0")
                    nc.vector.scalar_tensor_tensor(
                        out=A0[:], in0=s[:], scalar=-1.0, in1=A2[:],
                        op0=OP.mult, op1=OP.add)
                    nc.vector.tensor_scalar(
                        out=A0[:], in0=A0[:], scalar1=1.0, scalar2=None,
                        op0=OP.add)
                    axes.append((A0, A1, A2))
                    bases.append((base, sig))
                wA.append(axes)

                # gather row index (one per pixel):
                # row = (sx+6)*40 + (sy-4-(h0-10)) = 40*sx + sy + (246-h0)
                (bx, sx), (by, sy) = bases[0], bases[1]
                sxf = scrp.tile([128, NRB], F32, tag=f"sx{wi}",
                                name=f"sx{wi}")
                nc.vector.tensor_sub(sxf[:], bx[:], sx[:])
                syf = scrp.tile([128, NRB], F32, tag=f"sy{wi}",
                                name=f"sy{wi}")
                nc.vector.tensor_sub(syf[:], by[:], sy[:])
                r0 = scrp.tile([128, NRB], F32, tag=f"r0{wi}", name=f"r0{wi}")
                nc.vector.tensor_scalar(
                    out=r0[:], in0=sxf[:], scalar1=float(NY),
                    scalar2=y0con, op0=OP.mult, op1=OP.add)
                nc.vector.tensor_tensor(
                    out=r0[:], in0=r0[:], in1=syf[:], op=OP.add)
                ix16 = scrp.tile([128, NRB], I16, tag=f"ix16{wi}",
                                 name=f"ix16{wi}")
                nc.vector.tensor_copy(ix16[:], r0[:])

                # wrap roundtrip: idx i = rb*128 + p  ->
                # iscr[wi][p%16, rb*8 + p//16]
                nc.sync.dma_start(
                    out=mkap(iscr_d, [[1, 8], [WSLOT, 16], [8, NRB]],
                             offset_elems=wi * 16 * WSLOT),
                    in_=ix16[:])

                # masks for this warp -> multiply into maskC (du-major)
                mx = scrp.tile([128, NRB, ND], F32, tag=f"mx{wi}",
                               name=f"mx{wi}")
                my = scrp.tile([128, NRB, ND], F32, tag=f"my{wi}",
                               name=f"my{wi}")
                for (mt, v, mgt, lim) in ((mx, vx, mgx, float(W)),
                                          (my, vy, mgy, float(H))):
                    pos = scrp.tile([128, NRB, ND], F32, tag=f"pos{wi}",
                                    name=f"pos{wi}{lim}")
                    nc.vector.tensor_tensor(
                        out=pos[:],
                        in0=v[:].unsqueeze(-1).broadcast_to([128, NRB, ND]),
                        in1=mgt[:, wi * ND:(wi + 1) * ND].unsqueeze(1)
                            .broadcast_to([128, NRB, ND]),
                        op=OP.add)
                    t = scrp.tile([128, NRB, ND], F32, tag=f"mt{wi}",
                                  name=f"mt{wi}{lim}")
                    nc.vector.tensor_scalar(
                        out=t[:], in0=pos[:], scalar1=-1.0, scalar2=lim,
                        op0=OP.mult, op1=OP.add)
                    nc.vector.tensor_scalar(
                        out=pos[:], in0=pos[:], scalar1=1.0, scalar2=None,
                        op0=OP.add)
                    nc.vector.tensor_tensor(
                        out=t[:], in0=t[:], in1=pos[:], op=OP.min)
                    nc.vector.tensor_scalar(
                        out=mt[:], in0=t[:], scalar1=0.0, scalar2=1.0,
                        op0=OP.max, op1=OP.min)
                mw = scrp.tile([128, NRB, ND, ND], F32, tag=f"mw{wi}",
                               name=f"mw{wi}")
                # du-major: mw[n, du, dv] = mx[du] * my[dv]
                nc.vector.tensor_tensor(
                    out=mw[:],
                    in0=mx[:].unsqueeze(-1).broadcast_to([128, NRB, ND, ND]),
                    in1=my[:].unsqueeze(2).broadcast_to([128, NRB, ND, ND]),
                    op=OP.mult)
                nc.vector.tensor_scalar(
                    out=mw[:], in0=mw[:], scalar1=0.999, scalar2=None,
                    op0=OP.is_ge)
                if wi == 0:
                    nc.vector.tensor_copy(
                        maskC[:], mw[:].rearrange("p n a b -> p n (a b)"))
                else:
                    nc.vector.tensor_mul(
                        maskC[:], maskC[:],
                        mw[:].rearrange("p n a b -> p n (a b)"))

            # read back wrapped idxs
            wrs = []
            for wi in range(2):
                wr = fldp.tile([128, WSLOT], I16, tag=f"wr{wi}",
                               name=f"wr{wi}")
                src = iscr_d.ap()[wi]
                nc.sync.dma_start(
                    out=wr[:],
                    in_=src.unsqueeze(0).broadcast_to([8, 16, WSLOT]))
                wrs.append(wr)

            _lowpri.__exit__(None, None, None)
            normcm.__exit__(None, None, None)
            scrcm.__exit__(None, None, None)

            # ------------ Phase C: per-rb loop ------------------------------
            tabs = (tab1_d, tab2_d)
            with (
                tc.tile_pool(name="oap", bufs=2) as oap,
                tc.tile_pool(name="pp", bufs=2) as pp,
                tc.tile_pool(name="typ", bufs=1) as typ,
                tc.tile_pool(name="txp", bufs=1) as txp,
                tc.tile_pool(name="pyp", bufs=2) as pyp,
                tc.tile_pool(name="fwp", bufs=2) as fwp,
                tc.tile_pool(name="dotp", bufs=1) as dotp,
            ):
                Ptiles = {}

                def emit_gathers(rb):
                    for wi in range(2):
                        P = pp.tile([128, 1, GE], BF16, tag=f"P{wi}",
                                    name=f"P{wi}_{rb}")
                        tv = mkap(tabs[wi], [[ROWE, NTROW], [1, GE]])
                        nc.gpsimd.dma_gather(
                            out_ap=P[:],
                            in_ap=tv,
                            idxs_ap=wrs[wi][:, rb * 8:(rb + 1) * 8],
                            num_idxs=128,
                            num_idxs_reg=128,
                            elem_size=GE,
                            elem_step=ROWE,
                            single_packet=False,
                            queue_num=wi,
                        )
                        Ptiles[(rb, wi)] = P

                pend = []

                def emit_dot(rb, fw):
                    # split along the x-unit axis at the Pool o-chunk
                    # boundary (units 0..3 | 4..8) so each piece can start
                    # as soon as its fw chunks land, instead of waiting for
                    # the whole x-pass.
                    prod = dotp.tile([128, R, C], BF16, tag="prod")
                    T1 = dotp.tile([128, R, C // 2], BF16, tag="T1")
                    T2 = dotp.tile([128, R, C // 4], BF16, tag="T2")
                    T3 = dotp.tile([128, R, C // 8], BF16, tag="T3")
                    T4 = dotp.tile([128, R, C // 16], BF16, tag="T4")
                    oacc = oap.tile([128, R], F32, tag="oacc")
                    for (c0, c1) in PROD_SPLIT:
                        r0, r1 = c0 * ND, c1 * ND
                        nc.vector.tensor_mul(
                            prod[:, r0:r1, :].rearrange("p r c -> p (r c)"),
                            fw[:, 0, c0:c1].rearrange(
                                "p u v c -> p (u v c)"),
                            fw[:, 1, c0:c1].rearrange(
                                "p u v c -> p (u v c)"))
                    for (c0, c1) in DOT_SPLIT:
                        r0, r1 = c0 * ND, c1 * ND
                        (nc.gpsimd if T1_POOL else nc.vector).tensor_tensor(
                            out=T1[:, r0:r1, :], in0=prod[:, r0:r1, 0:24],
                            in1=prod[:, r0:r1, 24:48], op=OP.add)
                        nc.gpsimd.tensor_tensor(
                            out=T2[:, r0:r1, :], in0=T1[:, r0:r1, 0:12],
                            in1=T1[:, r0:r1, 12:24], op=OP.add)
                        nc.gpsimd.tensor_tensor(
                            out=T3[:, r0:r1, :], in0=T2[:, r0:r1, 0:6],
                            in1=T2[:, r0:r1, 6:12], op=OP.add)
                        nc.gpsimd.tensor_tensor(
                            out=T4[:, r0:r1, :], in0=T3[:, r0:r1, 0:3],
                            in1=T3[:, r0:r1, 3:6], op=OP.add)
                        nc.vector.tensor_reduce(
                            oacc[:, r0:r1], T4[:, r0:r1, :],
                            axis=mybir.AxisListType.X, op=OP.add)
                        nc.vector.tensor_mul(
                            oacc[:, r0:r1], oacc[:, r0:r1],
                            maskC[:, rb, r0:r1])
                    nc.sync.dma_start(
                        out=mkap(out_d, [[NRB * R, 128], [1, R]],
                                 offset_elems=rb * R),
                        in_=oacc[:])

                emit_gathers(0)
                emit_gathers(1)
                for rb in range(NRB):
                    PF = Ptiles.pop((rb, 0))
                    PB = Ptiles.pop((rb, 1))
                    # ---- y-pass: per warp, 9 units -> Py[128, 2, 9, 528] --
                    Py = pyp.tile([128, 2, ND, NCOLS * C], BF16, tag="Py")
                    for wi in range(2):
                        P = (PF, PB)[wi]
                        ty0 = typ.tile([128, ND, NCOLS * C], BF16,
                                       tag="ty0")
                        ty1 = typ.tile([128, ND, NCOLS * C], BF16,
                                       tag="ty1")

                        def y_ins(idv, P=P, wi=wi):
                            q = idv if wi == 0 else (ND - 1 - idv)
                            return tuple(
                                P[:, 0, (q + j) * ROWE:
                                  (q + j) * ROWE + NCOLS * C]
                                for j in range(3))

                        def y_w(idv, rb=rb, wi=wi):
                            (A0y, A1y, A2y) = wA[wi][1]
                            return (A0y[:, rb, idv:idv + 1],
                                    A1y[:, rb, idv:idv + 1],
                                    A2y[:, rb, idv:idv + 1])

                        def y_out(u0, u1, Py=Py, wi=wi):
                            return Py[:, wi, u0:u1, :]

                        emit_pass(nc, ty0, ty1, YW, NCOLS * C,
                                  y_ins, y_w, y_out)

                    # ---- x-pass, warp-stage interleaved ------------------
                    # stages: [j0/j1 muls F,B] [s-adds F,B] [j2 muls F,B]
                    # [o-adds F,B] so ACT's mul stream never waits on adds.
                    fw = fwp.tile([128, 2, ND, ND, C], BF16, tag="fw")
                    txs = []
                    for wi in range(2):
                        tx0 = txp.tile([128, ND, ND * C], BF16,
                                       tag=f"tx0w{wi}", name=f"tx0w{wi}")
                        tx1 = txp.tile([128, ND, ND * C], BF16,
                                       tag=f"tx1w{wi}", name=f"tx1w{wi}")
                        txs.append((tx0, tx1))

                    xws = XWS_TAB.get(rb, XWS) if XWS_TAB else (
                        XWS if rb < NRB - 1 else XWS_LAST)

                    def x_mul(wi, idu, j, dst, rb=rb, Py=Py, xws=xws):
                        q = idu if wi == 0 else (ND - 1 - idu)
                        x = Py[:, wi, :, (q + j) * C:(q + j + 1) * C]
                        w = wA[wi][0][j][:, rb, idu:idu + 1]
                        if j in xws[wi][idu][0]:
                            nc.scalar.activation(dst, x, AF.Copy, scale=w)
                        else:
                            nc.vector.tensor_scalar(
                                out=dst, in0=x, scalar1=w, scalar2=None,
                                op0=OP.mult)

                    def x_adds(wi, which, fw=fw, xws=xws):
                        t0, t1 = txs[wi]
                        for (u0, u1, e) in _runs(xws[wi], which):
                            pc = (POOL_CHUNK_S if which == 's'
                                  else POOL_CHUNK_O)
                            step = pc if e == 'P' else (u1 - u0)
                            for c0 in range(u0, u1, step):
                                c1 = min(c0 + step, u1)
                                eng = (nc.vector if e == 'D'
                                       else nc.gpsimd)
                                sl = (slice(None), slice(c0, c1),
                                      slice(None))
                                if which == 's':
                                    out = t0[sl]
                                else:
                                    out = fw[:, wi, c0:c1, :, :].rearrange(
                                        "p u v c -> p u (v c)")
                                eng.tensor_tensor(out=out, in0=t0[sl],
                                                  in1=t1[sl], op=OP.add)

                    for wi in range(2):
                        for u in range(ND):
                            x_mul(wi, u, 0, txs[wi][0][:, u, :])
                            x_mul(wi, u, 1, txs[wi][1][:, u, :])
                    if len(pend) > 1:
                        emit_dot(*pend.pop(0))
                    for wi in range(2):
                        x_adds(wi, 's')
                    for wi in range(2):
                        for u in range(ND):
                            x_mul(wi, u, 2, txs[wi][1][:, u, :])
                    for wi in range(2):
                        x_adds(wi, 'o')

                    if rb + 2 < NRB:
                        emit_gathers(rb + 2)
                    pend.append((rb, fw))
                for args in pend:
                    emit_dot(*args)

            fldcm.__exit__(None, None, None)

    nc.compile()
    return nc


def make_in_maps(feature1, feature2, BM):
    f1 = np.asarray(feature1, dtype=np.float32)
    f2 = np.asarray(feature2, dtype=np.float32)
    bm = np.asarray(BM, dtype=np.float32)

    wio = np.arange(W, dtype=np.float32).reshape(128, 1)

    def padded_slice(f, b, h0):
        ys = np.clip(h0 - 10 + np.arange(NY), 0, H - 1)
        xs = np.clip(np.arange(Wp) - PADL, 0, W - 1)
        s = f[b][:, ys][:, :, xs]                 # [C, NY, Wp]
        s = np.ascontiguousarray(s.transpose(1, 2, 0)).reshape(NY * Wp, C)
        out = np.zeros((NPX, C), np.float32)
        out[:NY * Wp] = s
        return out

    mgx = np.zeros((128, 2 * ND), np.float32)
    mgy = np.zeros((128, 2 * ND), np.float32)
    gx = np.zeros((128, 2 * ND), np.float32)
    gy = np.zeros((128, 2 * ND), np.float32)
    d = LIN.astype(np.float64)
    for wi, sgn in ((0, 1.0), (1, -1.0)):
        gx[:, wi * ND:(wi + 1) * ND] = (sgn * d * (SW - 1.0)).astype(
            np.float32)[None, :]
        gy[:, wi * ND:(wi + 1) * ND] = (sgn * d * (SH - 1.0)).astype(
            np.float32)[None, :]
        mgx[:, wi * ND:(wi + 1) * ND] = (sgn * d * SW).astype(
            np.float32)[None, :]
        mgy[:, wi * ND:(wi + 1) * ND] = (sgn * d * SH).astype(
            np.float32)[None, :]

    in_maps = []
    for k in range(NCORES):
        b = k // 4
        h0 = 20 * (k % 4)
        hcon = np.broadcast_to(
            (h0 + np.arange(NRB)).astype(np.float32)[None, :],
            (128, NRB)).copy()
        # row = 40*sx + sy + (246 - h0)
        y0con = np.full((128, 1), np.float32(246 - h0), np.float32)
        cst = np.concatenate([
            wio, hcon, y0con, gx, gy, mgx, mgy,
            np.ascontiguousarray(bm[b, 0, h0:h0 + NRB, :].T),
            np.ascontiguousarray(bm[b, 1, h0:h0 + NRB, :].T),
        ], axis=1).astype(np.float32)
        assert cst.shape == (128, 134), cst.shape
        in_maps.append({
            "f1s": padded_slice(f1, b, h0),
            "f2s": padded_slice(f2, b, h0),
            "cst": np.ascontiguousarray(cst),
        })
    return in_maps


_NC_CACHE = {}


def get_program():
    if "nc" not in _NC_CACHE:
        _NC_CACHE["nc"] = build_program()
    return _NC_CACHE["nc"]


# permutation: our r' = du*9+dv  ->  reference r = dv*9+du
_PERM = np.array([(rp % ND) * ND + rp // ND for rp in range(R)])


def core_to_ref(co):
    """co [128(w), NRB, R'] -> [R, NRB, 128] in reference r order."""
    inv = np.empty(R, np.int64)
    inv[_PERM] = np.arange(R)
    return co.transpose(2, 1, 0)[inv]


def assemble_output(results):
    out = np.zeros((B_, R, H, W), np.float32)
    for k in range(NCORES):
        b = k // 4
        h0 = 20 * (k % 4)
        co = results[k]["out"].reshape(128, NRB, R)   # [w, hh, r']
        out[b, :, h0:h0 + NRB, :] = core_to_ref(co)
    return out


def kernel(feature1, feature2, BM):
    nc = get_program()
    in_maps = make_in_maps(feature1, feature2, BM)
    res = bass_utils.run_bass_kernel_spmd(
        nc, in_maps, core_ids=list(range(NCORES)))
    return assemble_output(res.results)


# revision 79
# speedup vs baseline: 1.0053x; 1.0002x over previous
"""Bass/TRN2 kernel for nn_BilateralCostVolume — patch-gather scheme v2.

Sharding: core k handles batch b = k//4, output rows h in [20*(k%4), +20).
Per core, per pixel, per warp (F: +displacement on f2n; B: -displacement on
f1n) gather an 11x11 patch from a DRAM table laid out [xwin][y] so the 11
patch rows per pixel are CONTIGUOUS (one gather descriptor per pixel,
elem_step=640, elem_size=7040).  All 81 displacements are then computed
on-chip with static 3-tap separable interpolation (carry folded into
per-pixel weights), channel dot, mask.

Engine split (tunable tables below): y-pass fully on DVE (tensor_scalar
muls run in 4x perf mode, tensor_tensor adds in 2x, merged across the 9
units of a warp), x-pass muls mostly on ACT with adds on Pool (chunked so
they start while the ACT mul stream runs; stage-interleaved across warps
so no engine waits on another's adds), dot tree split DVE/Pool.  The dot
of row-block rb-1 is emitted inside rb's body (software pipelining) and
gathers are prefetched two row-blocks ahead.  Norm squares run on ACT and
the normalize-multiply on Pool so the fp2->table DMA chain is not stuck
behind Phase B on DVE.

out[b, du*9+dv -> r=dv*9+du, h0+hh, w] = core_out[w, hh*81 + (du*9+dv)].
"""

import numpy as np

import concourse.bass as bass
import concourse.bacc as bacc
import concourse.mybir as mybir
import concourse.tile as tile
from concourse import bass_utils

B_, C, H, W = 2, 48, 80, 128
R = 81
ND = 9
MD = 4
SW = W / (W - 1.0)
SH = H / (H - 1.0)
TH_X = 4.0 * (SW - 1.0)
TH_Y = 4.0 * (SH - 1.0)
NCORES = 8
NRB = 20            # output rows per core
PADL = 10
NCOLS = 11          # cols per table row
N_XS = 138          # x starts
Wp = 148            # padded width
NY = 40             # table y rows
NK = 11             # patch rows per pixel
ROWE = 640          # elems per table row (bf16): 528 used + pad (1280 B)
GE = NK * ROWE      # 7040 gather elems per pixel (14080 B, mult of 256)
NPX = 6016          # padded pixel rows in fp dram (40*148=5920 -> 47*128)
NTROW = NY * N_XS   # 5520 table rows
WSLOT = NRB * 8     # 160 idx slots per warp (wrapped 16p x 8 per rb)

F32 = mybir.dt.float32
I32 = mybir.dt.int32
I16 = mybir.dt.int16
BF16 = mybir.dt.bfloat16
AF = mybir.ActivationFunctionType
OP = mybir.AluOpType
LIN = np.linspace(-MD, MD, ND)

# ---- per-unit strategy tables (per warp, 9 units each) -----------------
# Entry = (act_taps, s_add, o_add):
#   act_taps: tap indices (0..2) whose mul runs on ACT; rest DVE ts.
#   s_add / o_add: 'D' (DVE tensor_tensor) or 'P' (Pool tensor_tensor).
# Adds merge across contiguous unit runs sharing an engine; Pool runs are
# chunked (POOL_CHUNK units) so they can start while ACT muls stream.
YW = [((), 'D', 'D')] * 9
XW0 = ([((0, 1, 2), 'P', 'P')] * 2 + [((0, 1, 2), 'D', 'P')] * 6
       + [((), 'D', 'P')] * 1)
XW1 = ([((0, 1, 2), 'P', 'P')] * 2 + [((0, 1, 2), 'D', 'P')] * 6
       + [((), 'D', 'P')] * 1)
XWS = (XW0, XW1)
# last row-block drains with no following work to overlap: run warp 1's
# x-pass on DVE so ACT (warp 0) and DVE (warp 1) drain in parallel, and
# warp 0's unit-8 adds on DVE too (Pool's tail chunk otherwise straggles)
XWS_LAST = ([((0, 1, 2), 'P', 'P')] * 2 + [((0, 1, 2), 'D', 'P')] * 6
            + [((), 'D', 'D')] * 1,
            [((), 'D', 'D')] * 9)
# late row-blocks: unit-8's adds (Pool's 1-unit straggler chunk) on DVE
XWS_U8D = ([((0, 1, 2), 'P', 'P')] * 2 + [((0, 1, 2), 'D', 'P')] * 6
           + [((), 'D', 'D')] * 1,
           [((0, 1, 2), 'P', 'P')] * 2 + [((0, 1, 2), 'D', 'P')] * 6
           + [((), 'D', 'D')] * 1)
# rb18 intermediate: u8d plus warp1 units 7-8 adds on DVE
XWS_MID = (XWS_U8D[0],
           [((0, 1, 2), 'P', 'P')] * 2 + [((0, 1, 2), 'D', 'P')] * 5
           + [((0, 1, 2), 'D', 'D')] * 1 + [((), 'D', 'D')] * 1)
XWS_TAB = {NRB - 1: XWS_LAST, NRB - 2: XWS_MID,
           NRB - 3: XWS_U8D, NRB - 4: XWS_U8D}
T1_POOL = False       # first dot-tree level on Pool
POOL_CHUNK_S = 4
POOL_CHUNK_O = 4
DOT_SPLIT = ((0, 9),)
PROD_SPLIT = ((0, 9),)


def mkap(t, dims, offset_elems=0):
    """Overlapping/custom AP on a dram tensor: dims = [[stride, count], ...]."""
    import bass_rust
    a = t.ap().copy() if hasattr(t, "ap") else t.copy()
    a.ap = bass_rust.VecI64Pair([list(d) for d in dims])
    if offset_elems:
        a.offset = a.offset + offset_elems
    return a


def _runs(strat, which):
    """Yield (start, end, engine) runs of equal add-engine assignment."""
    idx = 1 if which == 's' else 2
    runs = []
    s = 0
    for u in range(1, len(strat) + 1):
        if u == len(strat) or strat[u][idx] != strat[s][idx]:
            runs.append((s, u, strat[s][idx]))
            s = u
    return runs


def emit_pass(nc, t0, t1, strat, n, ins_fn, w_fn, out_ap_fn, t2=None):
    """Generic 3-tap pass over nu units.

    With t2: all three muls emitted up front (no cross-engine stall on the
    3rd mul), then s-add t0+=t1, o-add out=t0+t2.
    Without t2: t1 is reused for the 3rd mul after the s-add consumed it.
    """
    nu = len(strat)

    def mul(u, j, dst):
        xs = ins_fn(u)
        ws = w_fn(u)
        if j in strat[u][0]:
            nc.scalar.activation(dst, xs[j], AF.Copy, scale=ws[j])
        else:
            nc.vector.tensor_scalar(
                out=dst, in0=xs[j], scalar1=ws[j], scalar2=None, op0=OP.mult)

    def adds(which, tlast, dst_fn):
        for (u0, u1, e) in _runs(strat, which):
            pc = POOL_CHUNK_S if which == 's' else POOL_CHUNK_O
            step = pc if e == 'P' else (u1 - u0)
            for c0 in range(u0, u1, step):
                c1 = min(c0 + step, u1)
                eng = nc.vector if e == 'D' else nc.gpsimd
                sl = (slice(None), slice(c0, c1), slice(None))
                eng.tensor_tensor(out=dst_fn(c0, c1, sl), in0=t0[sl],
                                  in1=tlast[sl], op=OP.add)

    if t2 is not None:
        for u in range(nu):
            mul(u, 0, t0[:, u, :])
            mul(u, 1, t1[:, u, :])
            mul(u, 2, t2[:, u, :])
        adds('s', t1, lambda c0, c1, sl: t0[sl])
        adds('o', t2, lambda c0, c1, sl: out_ap_fn(c0, c1))
    else:
        for u in range(nu):
            mul(u, 0, t0[:, u, :])
            mul(u, 1, t1[:, u, :])
        adds('s', t1, lambda c0, c1, sl: t0[sl])
        for u in range(nu):
            mul(u, 2, t1[:, u, :])
        adds('o', t1, lambda c0, c1, sl: out_ap_fn(c0, c1))


def build_program():
    nc = bacc.Bacc(
        "TRN2",
        target_bir_lowering=False,
        debug=False,
        enable_asserts=False,
        num_devices=NCORES,
        num_swdge_queues=2,
    )

    f1s_d = nc.dram_tensor("f1s", [NPX, C], F32, kind="ExternalInput")
    f2s_d = nc.dram_tensor("f2s", [NPX, C], F32, kind="ExternalInput")
    # constants: [wio 1 | hcon 20 | y0con 1 | gx 18 | gy 18 | mgx 18 |
    # mgy 18 | bmx 20 | bmy 20] = 134 cols
    cst_d = nc.dram_tensor("cst", [128, 134], F32, kind="ExternalInput")

    fp1_d = nc.dram_tensor("fp1", [NPX, C], BF16, kind="Internal")
    fp2_d = nc.dram_tensor("fp2", [NPX, C], BF16, kind="Internal")
    tab1_d = nc.dram_tensor("tab1", [NTROW + 16, ROWE], BF16, kind="Internal")
    tab2_d = nc.dram_tensor("tab2", [NTROW + 16, ROWE], BF16, kind="Internal")
    iscr_d = nc.dram_tensor("iscr", [2, 16, WSLOT], I16, kind="Internal")
    out_d = nc.dram_tensor("out", [128, NRB * R], F32, kind="ExternalOutput")

    with tile.TileContext(nc) as tc:
        with tc.tile_pool(name="const", bufs=1) as constp:
            eps = constp.tile([128, 1], F32)
            nc.gpsimd.memset(eps[:], 1e-6)
            cst = constp.tile([128, 134], F32)
            nc.sync.dma_start(out=cst[:], in_=cst_d.ap())
            wio = cst[:, 0:1]
            hcon = cst[:, 1:21]
            y0con = cst[:, 21:22]
            gx = cst[:, 22:40]
            gy = cst[:, 40:58]
            mgx = cst[:, 58:76]
            mgy = cst[:, 76:94]
            bmx = cst[:, 94:114]
            bmy = cst[:, 114:134]

            # pools opened before norm so norm can close first (LIFO),
            # after Phase B: closing it right after Phase A emits a drain
            # that would stall Phase B on the table-build DMAs.
            fldcm = tc.tile_pool(name="fld", bufs=1)
            fldp = fldcm.__enter__()
            scrcm = tc.tile_pool(name="scr", bufs=1)
            scrp = scrcm.__enter__()
            normcm = tc.tile_pool(name="norm", bufs=1)
            normp = normcm.__enter__()

            # ------------ Phase A: normalize -> fp dram -> table ------------
            if True:
                lds = []
                for i, fsrc in enumerate((f2s_d, f1s_d)):
                    ld = normp.tile([128, 47, C], F32, tag=f"ld{i}",
                                    name=f"ld{i}")
                    src = mkap(fsrc, [[47 * C, 128], [1, 47 * C]])
                    nc.sync.dma_start(
                        out=ld[:].rearrange("p i c -> p (i c)"), in_=src)
                    lds.append(ld)
                for ld, fdst, tabd in ((lds[0], fp2_d, tab1_d),
                                       (lds[1], fp1_d, tab2_d)):
                    # norm on ACT+Pool so DVE stays free for Phase B and
                    # the table chain is not delayed behind it
                    sq = normp.tile([128, 47, C], F32, tag="sq")
                    nc.scalar.square(sq[:], ld[:])
                    ssq = normp.tile([128, 47], F32, tag="ssq")
                    nc.vector.tensor_reduce(
                        ssq[:], sq[:], axis=mybir.AxisListType.X, op=OP.add)
                    nc.scalar.activation(ssq[:], ssq[:], AF.Sqrt, bias=eps[:])
                    nc.vector.reciprocal(ssq[:], ssq[:])
                    nf = normp.tile([128, 47, C], BF16, tag="nf")
                    nc.gpsimd.tensor_mul(
                        nf[:], ld[:],
                        ssq[:].unsqueeze(-1).broadcast_to([128, 47, C]))
                    dst = mkap(fdst, [[47 * C, 128], [1, 47 * C]])
                    nc.sync.dma_start(
                        out=dst, in_=nf[:].rearrange("p i c -> p (i c)"))
                    # table build: tab[xw*NY + y] row = fp[y, xw..xw+10, :]
                    # (on the scalar-engine DMA queue so the next feature's
                    # load is not stuck behind it on the sync queue)
                    tsrc = mkap(fdst, [[C, N_XS], [Wp * C, NY],
                                       [1, NCOLS * C]])
                    tdst = mkap(tabd, [[NY * ROWE, N_XS], [ROWE, NY],
                                       [1, NCOLS * C]])
                    nc.scalar.dma_start(out=tdst, in_=tsrc)

            # ------------ Phase B: fields ----------------------------------
            # deprioritized so the scheduler prefers the norm->table chain
            # that gates the first gather
            _lowpri = tc.high_priority(offset=-1000000)
            _lowpri.__enter__()
            wA = []   # wA[warp][axis][tap] -> [128, NRB, ND] f32
            maskC = fldp.tile([128, NRB, R], BF16)

            for wi, sgn in ((0, 1.0), (1, -1.0)):
                vx = scrp.tile([128, NRB], F32, tag=f"vx{wi}", name=f"vx{wi}")
                nc.vector.tensor_scalar(
                    out=vx[:], in0=bmx, scalar1=sgn, scalar2=wio,
                    op0=OP.mult, op1=OP.add)
                nc.vector.tensor_scalar(
                    out=vx[:], in0=vx[:], scalar1=float(SW), scalar2=-0.5,
                    op0=OP.mult, op1=OP.add)
                vy = scrp.tile([128, NRB], F32, tag=f"vy{wi}", name=f"vy{wi}")
                nc.vector.tensor_scalar(
                    out=vy[:], in0=bmy, scalar1=sgn, scalar2=None,
                    op0=OP.mult)
                nc.vector.tensor_add(vy[:], vy[:], hcon)
                nc.vector.tensor_scalar(
                    out=vy[:], in0=vy[:], scalar1=float(SH), scalar2=-0.5,
                    op0=OP.mult, op1=OP.add)

                axes = []
                bases = []
                for ax, (v, th, gt) in enumerate(
                        ((vx, TH_X, gx), (vy, TH_Y, gy))):
                    pfx = f"w{wi}a{ax}"
                    t2_ = lambda tg: scrp.tile([128, NRB], F32,
                                               tag=pfx + tg, name=pfx + tg)
                    xi = scrp.tile([128, NRB], I32, tag=pfx + "i",
                                   name=pfx + "i")
                    nc.vector.tensor_copy(xi[:], v[:])
                    xf = t2_("xf")
                    nc.vector.tensor_copy(xf[:], xi[:])
                    er = t2_("er")
                    nc.vector.tensor_tensor(
                        out=er[:], in0=xf[:], in1=v[:], op=OP.is_gt)
                    base = t2_("b")
                    nc.vector.tensor_sub(base[:], xf[:], er[:])
                    fx = t2_("fx")
                    nc.vector.tensor_sub(fx[:], v[:], base[:])
                    sig = t2_("sg")
                    nc.vector.tensor_scalar(
                        out=sig[:], in0=fx[:], scalar1=float(th),
                        scalar2=None, op0=OP.is_lt)
                    t3_ = lambda tg: scrp.tile([128, NRB, ND], F32,
                                               tag=pfx + tg, name=pfx + tg)
                    gb = gt[:, wi * ND:(wi + 1) * ND]
                    gbb = gb.unsqueeze(1).broadcast_to([128, NRB, ND])
                    fxb = fx[:].unsqueeze(-1).broadcast_to([128, NRB, ND])
                    sgb = sig[:].unsqueeze(-1).broadcast_to([128, NRB, ND])
                    phi = t3_("ph")
                    nc.vector.tensor_tensor(
                        out=phi[:], in0=fxb, in1=gbb, op=OP.add)
                    thr = t2_("th")
                    nc.vector.tensor_scalar(
                        out=thr[:], in0=sig[:], scalar1=-1.0, scalar2=1.0,
                        op0=OP.mult, op1=OP.add)
                    ep = t3_("ep")
                    nc.vector.tensor_tensor(
                        out=ep[:], in0=phi[:],
                        in1=thr[:].unsqueeze(-1).broadcast_to([128, NRB, ND]),
                        op=OP.is_ge)
                    om = t3_("om")
                    nc.vector.tensor_sub(om[:], phi[:], ep[:])
                    nc.vector.tensor_tensor(
                        out=om[:], in0=om[:], in1=sgb, op=OP.add)
                    # A0 = (1-ep)(1-om), A1 = ep+om-2ep*om, A2 = ep*om
                    A2 = fldp.tile([128, NRB, ND], F32, tag=pfx + "A2",
                                   name=pfx + "A2")
                    nc.vector.tensor_mul(A2[:], ep[:], om[:])
                    s = t3_("s")
                    nc.vector.tensor_add(s[:], ep[:], om[:])
                    A1 = fldp.tile([128, NRB, ND], F32, tag=pfx + "A1",
                                   name=pfx + "A1")
                    nc.vector.scalar_tensor_tensor(
                        out=A1[:], in0=A2[:], scalar=-2.0, in1=s[:],
                        op0=OP.mult, op1=OP.add)
                    A0 = fldp.tile([128, NRB, ND], F32, tag=pfx + "A0",
                                   name=pfx + "A0")
                    nc.vector.scalar_tensor_tensor(
                        out=A0[:], in0=s[:], scalar=-1.0, in1=A2[:],
                        op0=OP.mult, op1=OP.add)
                    nc.vector.tensor_scalar(
                        out=A0[:], in0=A0[:], scalar1=1.0, scalar2=None,
                        op0=OP.add)
                    axes.append((A0, A1, A2))
                    bases.append((base, sig))
                wA.append(axes)

                # gather row index (one per pixel):
                # row = (sx+6)*40 + (sy-4-(h0-10)) = 40*sx + sy + (246-h0)
                (bx, sx), (by, sy) = bases[0], bases[1]
                sxf = scrp.tile([128, NRB], F32, tag=f"sx{wi}",
                                name=f"sx{wi}")
                nc.vector.tensor_sub(sxf[:], bx[:], sx[:])
                syf = scrp.tile([128, NRB], F32, tag=f"sy{wi}",
                                name=f"sy{wi}")
                nc.vector.tensor_sub(syf[:], by[:], sy[:])
                r0 = scrp.tile([128, NRB], F32, tag=f"r0{wi}", name=f"r0{wi}")
                nc.vector.tensor_scalar(
                    out=r0[:], in0=sxf[:], scalar1=float(NY),
                    scalar2=y0con, op0=OP.mult, op1=OP.add)
                nc.vector.tensor_tensor(
                    out=r0[:], in0=r0[:], in1=syf[:], op=OP.add)
                ix16 = scrp.tile([128, NRB], I16, tag=f"ix16{wi}",
                                 name=f"ix16{wi}")
                nc.vector.tensor_copy(ix16[:], r0[:])

                # wrap roundtrip: idx i = rb*128 + p  ->
                # iscr[wi][p%16, rb*8 + p//16]
                nc.sync.dma_start(
                    out=mkap(iscr_d, [[1, 8], [WSLOT, 16], [8, NRB]],
                             offset_elems=wi * 16 * WSLOT),
                    in_=ix16[:])

                # masks for this warp -> multiply into maskC (du-major)
                mx = scrp.tile([128, NRB, ND], F32, tag=f"mx{wi}",
                               name=f"mx{wi}")
                my = scrp.tile([128, NRB, ND], F32, tag=f"my{wi}",
                               name=f"my{wi}")
                for (mt, v, mgt, lim) in ((mx, vx, mgx, float(W)),
                                          (my, vy, mgy, float(H))):
                    pos = scrp.tile([128, NRB, ND], F32, tag=f"pos{wi}",
                                    name=f"pos{wi}{lim}")
                    nc.vector.tensor_tensor(
                        out=pos[:],
                        in0=v[:].unsqueeze(-1).broadcast_to([128, NRB, ND]),
                        in1=mgt[:, wi * ND:(wi + 1) * ND].unsqueeze(1)
                            .broadcast_to([128, NRB, ND]),
                        op=OP.add)
                    t = scrp.tile([128, NRB, ND], F32, tag=f"mt{wi}",
                                  name=f"mt{wi}{lim}")
                    nc.vector.tensor_scalar(
                        out=t[:], in0=pos[:], scalar1=-1.0, scalar2=lim,
                        op0=OP.mult, op1=OP.add)
                    nc.vector.tensor_scalar(
                        out=pos[:], in0=pos[:], scalar1=1.0, scalar2=None,
                        op0=OP.add)
                    nc.vector.tensor_tensor(
                        out=t[:], in0=t[:], in1=pos[:], op=OP.min)
                    nc.vector.tensor_scalar(
                        out=mt[:], in0=t[:], scalar1=0.0, scalar2=1.0,
                        op0=OP.max, op1=OP.min)
                mw = scrp.tile([128, NRB, ND, ND], F32, tag=f"mw{wi}",
                               name=f"mw{wi}")
                # du-major: mw[n, du, dv] = mx[du] * my[dv]
                nc.vector.tensor_tensor(
                    out=mw[:],
                    in0=mx[:].unsqueeze(-1).broadcast_to([128, NRB, ND, ND]),
                    in1=my[:].unsqueeze(2).broadcast_to([128, NRB, ND, ND]),
                    op=OP.mult)
                nc.vector.tensor_scalar(
                    out=mw[:], in0=mw[:], scalar1=0.999, scalar2=None,
                    op0=OP.is_ge)
                if wi == 0:
                    nc.vector.tensor_copy(
                        maskC[:], mw[:].rearrange("p n a b -> p n (a b)"))
                else:
                    nc.vector.tensor_mul(
                        maskC[:], maskC[:],
                        mw[:].rearrange("p n a b -> p n (a b)"))

            # read back wrapped idxs
            wrs = []
            for wi in range(2):
                wr = fldp.tile([128, WSLOT], I16, tag=f"wr{wi}",
                               name=f"wr{wi}")
                src = iscr_d.ap()[wi]
                nc.sync.dma_start(
                    out=wr[:],
                    in_=src.unsqueeze(0).broadcast_to([8, 16, WSLOT]))
                wrs.append(wr)

            _lowpri.__exit__(None, None, None)
            normcm.__exit__(None, None, None)
            scrcm.__exit__(None, None, None)

            # ------------ Phase C: per-rb loop ------------------------------
            tabs = (tab1_d, tab2_d)
            with (
                tc.tile_pool(name="oap", bufs=2) as oap,
                tc.tile_pool(name="pp", bufs=2) as pp,
                tc.tile_pool(name="typ", bufs=1) as typ,
                tc.tile_pool(name="txp", bufs=1) as txp,
                tc.tile_pool(name="pyp", bufs=2) as pyp,
                tc.tile_pool(name="fwp", bufs=2) as fwp,
                tc.tile_pool(name="dotp", bufs=1) as dotp,
            ):
                Ptiles = {}

                def emit_gathers(rb):
                    for wi in range(2):
                        P = pp.tile([128, 1, GE], BF16, tag=f"P{wi}",
                                    name=f"P{wi}_{rb}")
                        tv = mkap(tabs[wi], [[ROWE, NTROW], [1, GE]])
                        nc.gpsimd.dma_gather(
                            out_ap=P[:],
                            in_ap=tv,
                            idxs_ap=wrs[wi][:, rb * 8:(rb + 1) * 8],
                            num_idxs=128,
                            num_idxs_reg=128,
                            elem_size=GE,
                            elem_step=ROWE,
                            single_packet=False,
                            queue_num=wi,
                        )
                        Ptiles[(rb, wi)] = P

                pend = []

                def emit_dot(rb, fw):
                    # split along the x-unit axis at the Pool o-chunk
                    # boundary (units 0..3 | 4..8) so each piece can start
                    # as soon as its fw chunks land, instead of waiting for
                    # the whole x-pass.
                    prod = dotp.tile([128, R, C], BF16, tag="prod")
                    T1 = dotp.tile([128, R, C // 2], BF16, tag="T1")
                    T2 = dotp.tile([128, R, C // 4], BF16, tag="T2")
                    T3 = dotp.tile([128, R, C // 8], BF16, tag="T3")
                    T4 = dotp.tile([128, R, C // 16], BF16, tag="T4")
                    oacc = oap.tile([128, R], F32, tag="oacc")
                    for (c0, c1) in PROD_SPLIT:
                        r0, r1 = c0 * ND, c1 * ND
                        nc.vector.tensor_mul(
                            prod[:, r0:r1, :].rearrange("p r c -> p (r c)"),
                            fw[:, 0, c0:c1].rearrange(
                                "p u v c -> p (u v c)"),
                            fw[:, 1, c0:c1].rearrange(
                                "p u v c -> p (u v c)"))
                    for (c0, c1) in DOT_SPLIT:
                        r0, r1 = c0 * ND, c1 * ND
                        (nc.gpsimd if T1_POOL else nc.vector).tensor_tensor(
                            out=T1[:, r0:r1, :], in0=prod[:, r0:r1, 0:24],
                            in1=prod[:, r0:r1, 24:48], op=OP.add)
                        nc.gpsimd.tensor_tensor(
                            out=T2[:, r0:r1, :], in0=T1[:, r0:r1, 0:12],
                            in1=T1[:, r0:r1, 12:24], op=OP.add)
                        nc.gpsimd.tensor_tensor(
                            out=T3[:, r0:r1, :], in0=T2[:, r0:r1, 0:6],
                            in1=T2[:, r0:r1, 6:12], op=OP.add)
                        nc.gpsimd.tensor_tensor(
                            out=T4[:, r0:r1, :], in0=T3[:, r0:r1, 0:3],
                            in1=T3[:, r0:r1, 3:6], op=OP.add)
                        nc.vector.tensor_reduce(
                            oacc[:, r0:r1], T4[:, r0:r1, :],
                            axis=mybir.AxisListType.X, op=OP.add)
                        nc.vector.tensor_mul(
                            oacc[:, r0:r1], oacc[:, r0:r1],
                            maskC[:, rb, r0:r1])
                    nc.sync.dma_start(
                        out=mkap(out_d, [[NRB * R, 128], [1, R]],
                                 offset_elems=rb * R),
                        in_=oacc[:])

                emit_gathers(0)
                emit_gathers(1)
                for rb in range(NRB):
                    PF = Ptiles.pop((rb, 0))
                    PB = Ptiles.pop((rb, 1))
                    # ---- y-pass: per warp, 9 units -> Py[128, 2, 9, 528] --
                    Py = pyp.tile([128, 2, ND, NCOLS * C], BF16, tag="Py")
                    for wi in range(2):
                        P = (PF, PB)[wi]
                        ty0 = typ.tile([128, ND, NCOLS * C], BF16,
                                       tag="ty0")
                        ty1 = typ.tile([128, ND, NCOLS * C], BF16,
                                       tag="ty1")

                        def y_ins(idv, P=P, wi=wi):
                            q = idv if wi == 0 else (ND - 1 - idv)
                            return tuple(
                                P[:, 0, (q + j) * ROWE:
                                  (q + j) * ROWE + NCOLS * C]
                                for j in range(3))

                        def y_w(idv, rb=rb, wi=wi):
                            (A0y, A1y, A2y) = wA[wi][1]
                            return (A0y[:, rb, idv:idv + 1],
                                    A1y[:, rb, idv:idv + 1],
                                    A2y[:, rb, idv:idv + 1])

                        def y_out(u0, u1, Py=Py, wi=wi):
                            return Py[:, wi, u0:u1, :]

                        emit_pass(nc, ty0, ty1, YW, NCOLS * C,
                                  y_ins, y_w, y_out)

                    # ---- x-pass, warp-stage interleaved ------------------
                    # stages: [j0/j1 muls F,B] [s-adds F,B] [j2 muls F,B]
                    # [o-adds F,B] so ACT's mul stream never waits on adds.
                    fw = fwp.tile([128, 2, ND, ND, C], BF16, tag="fw")
                    txs = []
                    for wi in range(2):
                        tx0 = txp.tile([128, ND, ND * C], BF16,
                                       tag=f"tx0w{wi}", name=f"tx0w{wi}")
                        tx1 = txp.tile([128, ND, ND * C], BF16,
                                       tag=f"tx1w{wi}", name=f"tx1w{wi}")
                        txs.append((tx0, tx1))

                    xws = XWS_TAB.get(rb, XWS) if XWS_TAB else (
                        XWS if rb < NRB - 1 else XWS_LAST)

                    def x_mul(wi, idu, j, dst, rb=rb, Py=Py, xws=xws):
                        q = idu if wi == 0 else (ND - 1 - idu)
                        x = Py[:, wi, :, (q + j) * C:(q + j + 1) * C]
                        w = wA[wi][0][j][:, rb, idu:idu + 1]
                        if j in xws[wi][idu][0]:
                            nc.scalar.activation(dst, x, AF.Copy, scale=w)
                        else:
                            nc.vector.tensor_scalar(
                                out=dst, in0=x, scalar1=w, scalar2=None,
                                op0=OP.mult)

                    def x_adds(wi, which, fw=fw, xws=xws):
                        t0, t1 = txs[wi]
                        for (u0, u1, e) in _runs(xws[wi], which):
                            pc = (POOL_CHUNK_S if which == 's'
                                  else POOL_CHUNK_O)
                            step = pc if e == 'P' else (u1 - u0)
                            for c0 in range(u0, u1, step):
                                c1 = min(c0 + step, u1)
                                eng = (nc.vector if e == 'D'
                                       else nc.gpsimd)
                                sl = (slice(None), slice(c0, c1),
                                      slice(None))
                                if which == 's':
                                    out = t0[sl]
                                else:
                                    out = fw[:, wi, c0:c1, :, :].rearrange(
                                        "p u v c -> p u (v c)")
                                eng.tensor_tensor(out=out, in0=t0[sl],
                                                  in1=t1[sl], op=OP.add)

                    for wi in range(2):
                        for u in range(ND):
                            x_mul(wi, u, 0, txs[wi][0][:, u, :])
                            x_mul(wi, u, 1, txs[wi][1][:, u, :])
                    if len(pend) > 1:
                        emit_dot(*pend.pop(0))
                    for wi in range(2):
                        x_adds(wi, 's')
                    for wi in range(2):
                        for u in range(ND):
                            x_mul(wi, u, 2, txs[wi][1][:, u, :])
                    for wi in range(2):
                        x_adds(wi, 'o')

                    if rb + 2 < NRB:
                        emit_gathers(rb + 2)
                    pend.append((rb, fw))
                for args in pend:
                    emit_dot(*args)

            fldcm.__exit__(None, None, None)

    nc.compile()
    return nc


def make_in_maps(feature1, feature2, BM):
    f1 = np.asarray(feature1, dtype=np.float32)
    f2 = np.asarray(feature2, dtype=np.float32)
    bm = np.asarray(BM, dtype=np.float32)

    wio = np.arange(W, dtype=np.float32).reshape(128, 1)

    def padded_slice(f, b, h0):
        ys = np.clip(h0 - 10 + np.arange(NY), 0, H - 1)
        xs = np.clip(np.arange(Wp) - PADL, 0, W - 1)
        s = f[b][:, ys][:, :, xs]                 # [C, NY, Wp]
        s = np.ascontiguousarray(s.transpose(1, 2, 0)).reshape(NY * Wp, C)
        out = np.zeros((NPX, C), np.float32)
        out[:NY * Wp] = s
        return out

    mgx = np.zeros((128, 2 * ND), np.float32)
    mgy = np.zeros((128, 2 * ND), np.float32)
    gx = np.zeros((128, 2 * ND), np.float32)
    gy = np.zeros((128, 2 * ND), np.float32)
    d = LIN.astype(np.float64)
    for wi, sgn in ((0, 1.0), (1, -1.0)):
        gx[:, wi * ND:(wi + 1) * ND] = (sgn * d * (SW - 1.0)).astype(
            np.float32)[None, :]
        gy[:, wi * ND:(wi + 1) * ND] = (sgn * d * (SH - 1.0)).astype(
            np.float32)[None, :]
        mgx[:, wi * ND:(wi + 1) * ND] = (sgn * d * SW).astype(
            np.float32)[None, :]
        mgy[:, wi * ND:(wi + 1) * ND] = (sgn * d * SH).astype(
            np.float32)[None, :]

    in_maps = []
    for k in range(NCORES):
        b = k // 4
        h0 = 20 * (k % 4)
        hcon = np.broadcast_to(
            (h0 + np.arange(NRB)).astype(np.float32)[None, :],
            (128, NRB)).copy()
        # row = 40*sx + sy + (246 - h0)
        y0con = np.full((128, 1), np.float32(246 - h0), np.float32)
        cst = np.concatenate([
            wio, hcon, y0con, gx, gy, mgx, mgy,
            np.ascontiguousarray(bm[b, 0, h0:h0 + NRB, :].T),
            np.ascontiguousarray(bm[b, 1, h0:h0 + NRB, :].T),
        ], axis=1).astype(np.float32)
        assert cst.shape == (128, 134), cst.shape
        in_maps.append({
            "f1s": padded_slice(f1, b, h0),
            "f2s": padded_slice(f2, b, h0),
            "cst": np.ascontiguousarray(cst),
        })
    return in_maps


_NC_CACHE = {}


def get_program():
    if "nc" not in _NC_CACHE:
        _NC_CACHE["nc"] = build_program()
    return _NC_CACHE["nc"]


# permutation: our r' = du*9+dv  ->  reference r = dv*9+du
_PERM = np.array([(rp % ND) * ND + rp // ND for rp in range(R)])


def core_to_ref(co):
    """co [128(w), NRB, R'] -> [R, NRB, 128] in reference r order."""
    inv = np.empty(R, np.int64)
    inv[_PERM] = np.arange(R)
    return co.transpose(2, 1, 0)[inv]


def assemble_output(results):
    out = np.zeros((B_, R, H, W), np.float32)
    for k in range(NCORES):
        b = k // 4
        h0 = 20 * (k % 4)
        co = results[k]["out"].reshape(128, NRB, R)   # [w, hh, r']
        out[b, :, h0:h0 + NRB, :] = core_to_ref(co)
    return out


def kernel(feature1, feature2, BM):
    nc = get_program()
    in_maps = make_in_maps(feature1, feature2, BM)
    res = bass_utils.run_bass_kernel_spmd(
        nc, in_maps, core_ids=list(range(NCORES)))
    return assemble_output(res.results)
